# revision 1
# baseline (speedup 1.0000x reference)
"""Trainium2 Bass kernel for a masked-attention block (MAB).

Computation (per batch element, all fp32):
    Q = X@Wq + bq ; K = Y@Wk + bk ; V = Y@Wv + bv
    logits = per-head Qh@Kh^T / 32, masked keys -> -inf, softmax over keys
    attn   = A @ Vh (concat heads)
    O1 = LN(Q + attn; g1,b1)
    O  = LN(O1 + relu(O1@Wo + bo); g2,b2)

Sharding: pure data-parallel, one batch element per NeuronCore (B=8 = 8 cores).

On-device dataflow is "feature-major": activations live in SBUF transposed
([model_dim -> 8x128 partitions, token -> free]).  With weights in natural
layout every matmul chains without any transposes:
    actT_out[n, t] = sum_d W[d, n] * actT_in[d, t]   (lhsT=W, rhs=actT_in)
Attention also chains: logitsT[k, q] from (lhsT=KT_h, rhs=QT_h) single
128-contraction; exp on ACT (mask folded in as a per-partition bias);
AV from (lhsT=V_natural, rhs=expT).  The softmax denominator and the
LayerNorm stats are partition-dim reductions done with all-ones stationary
matmuls (which also broadcast the result across partitions for free).
All matmuls use float32r (FP22 truncation) which runs at full PE rate for
moving free-dim >= 256.

The host transposes X/Y on the way in and the output on the way out, and
converts the bool mask into an additive f32 bias (0 / -1e4).
"""

import math
import numpy as np
from contextlib import ExitStack

import concourse.bass as bass
import concourse.mybir as mybir
import concourse.tile as tile
from concourse import bacc
from concourse.bass_utils import run_bass_kernel_spmd

P = 128
NX = 1024
NY = 1024
DIM = 1024
H = 8
KO = DIM // P          # 8 partition sub-tiles of the model dim
QC = 512               # moving-operand chunk (fp32 max free dim)
NQC = NX // QC         # 2
F32 = mybir.dt.float32
F32R = mybir.dt.float32r
BF16 = mybir.dt.bfloat16
# ldw-opt dedupes adjacent same-stationary LDWEIGHTS, but it is disabled in
# every production compile config here and we could not A/B-verify it on
# hardware before the time budget ran out — keep it off.
ENABLE_LDW_OPT = False
AF = mybir.ActivationFunctionType
ALU = mybir.AluOpType
SCALE = 1.0 / 32.0     # 1/sqrt(DIM)
EPS = 1e-5


def _r(ap):
    return ap.bitcast(F32R)


_LDW_PATCHED = False


def _patch_ldw_opt():
    """walrus ships with --enable-ldw-opt=false hardcoded; with our loop
    order same-stationary matmuls are adjacent, so deduping LDWEIGHTS is a
    large PE win.  Rewrite the flag on the walrus command line."""
    global _LDW_PATCHED
    if _LDW_PATCHED or not ENABLE_LDW_OPT:
        return
    import concourse.bass_utils as _bu
    _orig = _bu.run_command

    def _run_command(argv, **kwargs):
        argv = ["--enable-ldw-opt=true" if a == "--enable-ldw-opt=false" else a
                for a in argv]
        return _orig(argv, **kwargs)

    _bu.run_command = _run_command
    _LDW_PATCHED = True


def _build():
    _patch_ldw_opt()
    nc = bacc.Bacc("TRN2", target_bir_lowering=False, debug=False,
                   enable_asserts=False)

    # ---- DRAM I/O (per-core shapes) ----
    XT = nc.dram_tensor("XT", [DIM, NX], F32, kind="ExternalInput").ap()
    YT = nc.dram_tensor("YT", [DIM, NY], F32, kind="ExternalInput").ap()
    MB = nc.dram_tensor("MB", [NY], F32, kind="ExternalInput").ap()
    Wd = {}
    for w in ("Wq", "Wk", "Wv", "Wo"):
        Wd[w] = nc.dram_tensor(w, [DIM, DIM], F32, kind="ExternalInput").ap()
    Vecs = {}
    for vname in ("bq", "bk", "bv", "bo", "g1", "b1", "g2", "b2"):
        Vecs[vname] = nc.dram_tensor(vname, [DIM], F32, kind="ExternalInput").ap()
    OT = nc.dram_tensor("OT", [DIM, NX], F32, kind="ExternalOutput").ap()

    xt3 = XT.rearrange("(ko p) q -> p ko q", p=P)
    yt3 = YT.rearrange("(ko p) q -> p ko q", p=P)
    wq3 = Wd["Wq"].rearrange("(ko p) d -> p ko d", p=P)
    wk3 = Wd["Wk"].rearrange("(ko p) d -> p ko d", p=P)
    wv3 = Wd["Wv"].rearrange("(ko p) d -> p ko d", p=P)
    wo3 = Wd["Wo"].rearrange("(ko p) d -> p ko d", p=P)
    ot3 = OT.rearrange("(do p) q -> p do q", p=P)

    with tile.TileContext(nc) as tc:
        with ExitStack() as octx:
            const = octx.enter_context(tc.tile_pool(name="const", bufs=1))
            actp = octx.enter_context(tc.tile_pool(name="act", bufs=3))

            # ---- constants ----
            # walrus requires every writer of an fp32r-matmul operand to have
            # an fp32r-tagged output AP; memset can't write f32r, so round
            # the ones through a copy
            ones128 = const.tile([P, P], F32, tag="ones", name="ones128")
            ones_tmp = const.tile([P, P], F32, tag="onest", name="ones_tmp")
            nc.vector.memset(ones_tmp, 1.0)
            nc.vector.tensor_copy(_r(ones128), ones_tmp)
            ones_bf = const.tile([P, P], BF16, tag="onesbf", name="ones_bf")
            nc.vector.memset(ones_bf, 1.0)
            eps_sb = const.tile([P, 1], F32, tag="eps", name="eps_sb")
            nc.vector.memset(eps_sb, EPS)

            def vec_pko(name):
                t = const.tile([P, KO], F32, tag=f"v_{name}", name=f"{name}_sb")
                nc.sync.dma_start(t, Vecs[name].rearrange("(ko p) -> p ko", p=P))
                return t

            mb_sb = const.tile([P, KO], F32, tag="v_mb", name="mb_sb")
            nc.sync.dma_start(mb_sb, MB.rearrange("(ko p) -> p ko", p=P))
            bq_sb = vec_pko("bq")
            bk_sb = vec_pko("bk")
            bo_sb = vec_pko("bo")
            g1_sb = vec_pko("g1")
            b1_sb = vec_pko("b1")
            g2_sb = vec_pko("g2")
            b2_sb = vec_pko("b2")
            bv_sb = const.tile([1, DIM], F32, tag="v_bv", name="bv_sb")
            nc.sync.dma_start(_r(bv_sb),
                              _r(Vecs["bv"].rearrange("(one n) -> one n", one=1)))

            # ---- big feature-major activation tiles (rotating slots) ----
            qt = actp.tile([P, KO, NX], F32, tag="big", name="qt")
            ktm = actp.tile([P, KO, NY], F32, tag="big", name="ktm")
            vm = actp.tile([P, KO, DIM], BF16, tag="big", name="vm")

            # ================= Phase 1: Q, K, V projections =================
            with tc.tile_pool(name="io", bufs=1) as iop, \
                 tc.tile_pool(name="w1", bufs=2) as wp, \
                 tc.tile_pool(name="gp1", bufs=8, space="PSUM") as pp:
                xt = iop.tile([P, KO, NX], F32, tag="xt", name="xt")
                yt = iop.tile([P, KO, NY], F32, tag="yt", name="yt")
                for k in range(KO):
                    nc.sync.dma_start(_r(xt[:, k, :]), _r(xt3[:, k, :]))
                for k in range(KO):
                    nc.sync.dma_start(_r(yt[:, k, :]), _r(yt3[:, k, :]))

                def proj_featmajor(w3, rhs_sb, out_sb, bias_sb, label):
                    # out_sb[p, do, q] (+= bias[do*128+p]) = sum_k W[k, d] rhs[k, q]
                    # qc innermost: both uses of each stationary tile are
                    # back-to-back so ldw-opt can dedupe the LDWEIGHTS
                    for dg in range(2):
                        wt = wp.tile([P, KO, QC], F32, tag="w", name=f"w_{label}{dg}")
                        for k in range(KO):
                            nc.sync.dma_start(_r(wt[:, k, :]),
                                              _r(w3[:, k, dg * QC:(dg + 1) * QC]))
                        for d4 in range(4):
                            pss = [pp.tile([P, QC], F32, tag="ps",
                                           name=f"ps_{label}{dg}{d4}{qc}")
                                   for qc in range(NQC)]
                            for k in range(KO):
                                for qc in range(NQC):
                                    qs = slice(qc * QC, (qc + 1) * QC)
                                    nc.tensor.matmul(
                                        pss[qc],
                                        lhsT=_r(wt[:, k, d4 * P:(d4 + 1) * P]),
                                        rhs=_r(rhs_sb[:, k, qs]),
                                        start=(k == 0), stop=(k == KO - 1))
                            do = dg * 4 + d4
                            for qc in range(NQC):
                                qs = slice(qc * QC, (qc + 1) * QC)
                                nc.scalar.activation(
                                    _r(out_sb[:, do, qs]), pss[qc], AF.Identity,
                                    bias=bias_sb[:, do:do + 1], scale=1.0)

                proj_featmajor(wq3, xt, qt, bq_sb, "q")
                proj_featmajor(wk3, yt, ktm, bk_sb, "k")

                # V in natural (token-major) layout: V[y, n] = sum_k Y[y,k] Wv[k,n]
                # (bf16 output — only consumed by the AV matmul).  ng innermost
                # so each yt stationary tile is used twice back-to-back.
                wts = []
                for ng in range(2):
                    wt = wp.tile([P, KO, QC], F32, tag="w", name=f"w_v{ng}")
                    for k in range(KO):
                        nc.sync.dma_start(_r(wt[:, k, :]),
                                          _r(wv3[:, k, ng * QC:(ng + 1) * QC]))
                    wts.append(wt)
                for yo in range(KO):
                    pss = [pp.tile([P, QC], F32, tag="ps", name=f"ps_v{yo}{ng}")
                           for ng in range(2)]
                    for k in range(KO):
                        for ng in range(2):
                            nc.tensor.matmul(
                                pss[ng],
                                lhsT=_r(yt[:, k, yo * P:(yo + 1) * P]),
                                rhs=_r(wts[ng][:, k, :]),
                                start=(k == 0), stop=False)
                    for ng in range(2):
                        ns = slice(ng * QC, (ng + 1) * QC)
                        # fold per-free-dim bias bv with a K=1 ones matmul
                        nc.tensor.matmul(
                            pss[ng], lhsT=_r(ones128[0:1, :]), rhs=_r(bv_sb[:, ns]),
                            start=False, stop=True)
                        nc.scalar.copy(vm[:, yo, ns], pss[ng])

            # ================= Phase 2: attention =================
            with tc.tile_pool(name="zp", bufs=1) as zp:
                zt = zp.tile([P, KO, NX], F32, tag="z", name="zt")

                with tc.tile_pool(name="exp", bufs=20) as ep, \
                     tc.tile_pool(name="rcp", bufs=2) as rp, \
                     tc.tile_pool(name="lgp", bufs=2, space="PSUM") as lgp, \
                     tc.tile_pool(name="avp", bufs=1, space="PSUM") as avp, \
                     tc.tile_pool(name="rlp", bufs=1, space="PSUM") as rlp:

                    def logits_exp(h):
                        # logitsT[k, q] = sum_d KT_h[d, k] QT_h[d, q]; exp with
                        # mask bias per key (partition) and 1/32 scale.  The
                        # logits psum tile spans 2 banks so one ACT op covers
                        # the whole [128, 1024] key-slice.  exp output is bf16
                        # (feeds only the bf16 AV/denominator matmuls).
                        et = [ep.tile([P, NY], BF16, tag="exp", name=f"et{h}_{k}")
                              for k in range(KO)]
                        for kt in range(KO):
                            pl = lgp.tile([P, NX], F32, tag="lg",
                                          name=f"pl{h}{kt}")
                            for qc in range(NQC):
                                qs = slice(qc * QC, (qc + 1) * QC)
                                nc.tensor.matmul(
                                    pl[:, qs],
                                    lhsT=_r(ktm[:, h, kt * P:(kt + 1) * P]),
                                    rhs=_r(qt[:, h, qs]),
                                    start=True, stop=True)
                            nc.scalar.activation(
                                et[kt], pl, AF.Exp,
                                bias=mb_sb[:, kt:kt + 1], scale=SCALE)
                        return et

                    def denom_av(h, et):
                        # softmax denominator: accumulate the all-ones matmul
                        # over the 8 key sub-tiles -> partition-reduction AND
                        # broadcast in one shot (also keeps PE warm here)
                        pr = rlp.tile([P, NX], F32, tag="rl", name=f"pr{h}")
                        for kt in range(KO):
                            for qc in range(NQC):
                                qs = slice(qc * QC, (qc + 1) * QC)
                                nc.tensor.matmul(
                                    pr[:, qs], lhsT=ones_bf,
                                    rhs=et[kt][:, qs],
                                    start=(kt == 0), stop=(kt == KO - 1))
                        rc = rp.tile([P, NX], F32, tag="rc", name=f"rc{h}")
                        nc.vector.reciprocal_approx_fast(rc, pr)
                        # attnT_h[d, q] = sum_k V[k, d_h] expT[k, q]; then
                        # normalize by the softmax denom and add the Q residual
                        pa = avp.tile([P, NX], F32, tag="av", name=f"pa{h}")
                        for kt in range(KO):
                            for qc in range(NQC):
                                qs = slice(qc * QC, (qc + 1) * QC)
                                nc.tensor.matmul(
                                    pa[:, qs],
                                    lhsT=vm[:, kt, h * P:(h + 1) * P],
                                    rhs=et[kt][:, qs],
                                    start=(kt == 0), stop=(kt == KO - 1))
                        nc.vector.tensor_mul(_r(zt[:, h, :]), pa, rc)
                        nc.vector.tensor_add(_r(zt[:, h, :]), zt[:, h, :],
                                             qt[:, h, :])

                    # software pipeline: head h's logits/exp (PE+ACT) run while
                    # head h-1's denominator+AV (PE) wait on h-1's exp -> PE
                    # never idles long enough for HAM to re-throttle
                    prev = None
                    for h in range(H):
                        et = logits_exp(h)
                        if prev is not None:
                            denom_av(h - 1, prev)
                        prev = et
                    denom_av(H - 1, prev)

                # ---- LayerNorm over the model dim (partition direction) ----
                def layernorm(in_sb, sqp, stp, spp, emit_out):
                    for qc in range(NQC):
                        qs = slice(qc * QC, (qc + 1) * QC)
                        pmu = spp.tile([P, QC], F32, tag="pmu", name=f"pmu{qc}")
                        ps2 = spp.tile([P, QC], F32, tag="ps2", name=f"ps2{qc}")
                        for do in range(KO):
                            nc.tensor.matmul(pmu, lhsT=_r(ones128),
                                             rhs=_r(in_sb[:, do, qs]),
                                             start=(do == 0), stop=(do == KO - 1))
                        for do in range(KO):
                            sq = sqp.tile([P, QC], F32, tag="sq", name=f"sq{qc}{do}")
                            nc.vector.tensor_mul(_r(sq), in_sb[:, do, qs],
                                                 in_sb[:, do, qs])
                            nc.tensor.matmul(ps2, lhsT=_r(ones128), rhs=_r(sq),
                                             start=(do == 0), stop=(do == KO - 1))
                        mu = stp.tile([P, QC], F32, tag="mu", name=f"mu{qc}")
                        nc.vector.tensor_scalar_mul(mu, pmu, 1.0 / DIM)
                        msq = stp.tile([P, QC], F32, tag="msq", name=f"msq{qc}")
                        nc.vector.tensor_mul(msq, mu, mu)
                        sd = stp.tile([P, QC], F32, tag="sd", name=f"sd{qc}")
                        nc.vector.scalar_tensor_tensor(
                            sd, ps2, 1.0 / DIM, msq,
                            op0=ALU.mult, op1=ALU.subtract)
                        nc.scalar.activation(sd, sd, AF.Sqrt, bias=eps_sb, scale=1.0)
                        rsig = stp.tile([P, QC], F32, tag="rsig", name=f"rsig{qc}")
                        nc.vector.reciprocal_approx_fast(rsig, sd)
                        mrs = stp.tile([P, QC], F32, tag="mrs", name=f"mrs{qc}")
                        nc.vector.tensor_mul(mrs, mu, rsig)
                        for do in range(KO):
                            t = sqp.tile([P, QC], F32, tag="t", name=f"t{qc}{do}")
                            nc.vector.tensor_mul(t, in_sb[:, do, qs], rsig)
                            nc.vector.tensor_sub(t, t, mrs)
                            emit_out(do, qs, t)

                # LN1 -> o1t (feature-major)
                with tc.tile_pool(name="sq1", bufs=3) as sqp1, \
                     tc.tile_pool(name="st1", bufs=2) as stp1, \
                     tc.tile_pool(name="sp1", bufs=2, space="PSUM") as spp1:
                    o1t = actp.tile([P, KO, NX], F32, tag="big", name="o1t")

                    def emit_o1(do, qs, t):
                        nc.vector.tensor_scalar(
                            _r(o1t[:, do, qs]), t,
                            scalar1=g1_sb[:, do:do + 1],
                            scalar2=b1_sb[:, do:do + 1],
                            op0=ALU.mult, op1=ALU.add)

                    layernorm(zt, sqp1, stp1, spp1, emit_o1)

            # ================= Phase 3: output proj + LN2 =================
            with tc.tile_pool(name="w3", bufs=2) as wp3, \
                 tc.tile_pool(name="sq2", bufs=4) as sqp2, \
                 tc.tile_pool(name="st2", bufs=2) as stp2, \
                 tc.tile_pool(name="out", bufs=4) as outp, \
                 tc.tile_pool(name="gp3", bufs=4, space="PSUM") as pp3, \
                 tc.tile_pool(name="sp2", bufs=2, space="PSUM") as spp2:
                z2t = actp.tile([P, KO, NX], F32, tag="big", name="z2t")
                # HT[n, q] = sum_d Wo[d, n] O1T[d, q];  z2 = o1 + relu(H + bo)
                for ng in range(2):
                    wt = wp3.tile([P, KO, QC], F32, tag="w", name=f"w_o{ng}")
                    for k in range(KO):
                        nc.sync.dma_start(_r(wt[:, k, :]),
                                          _r(wo3[:, k, ng * QC:(ng + 1) * QC]))
                    for qc in range(NQC):
                        qs = slice(qc * QC, (qc + 1) * QC)
                        for n4 in range(4):
                            ps = pp3.tile([P, QC], F32, tag="ps",
                                          name=f"ps_o{ng}{qc}{n4}")
                            for k in range(KO):
                                nc.tensor.matmul(
                                    ps,
                                    lhsT=_r(wt[:, k, n4 * P:(n4 + 1) * P]),
                                    rhs=_r(o1t[:, k, qs]),
                                    start=(k == 0), stop=(k == KO - 1))
                            no = ng * 4 + n4
                            ht = sqp2.tile([P, QC], F32, tag="ht",
                                           name=f"ht{ng}{qc}{n4}")
                            nc.scalar.activation(ht, ps, AF.Relu,
                                                 bias=bo_sb[:, no:no + 1], scale=1.0)
                            nc.vector.tensor_add(_r(z2t[:, no, qs]), ht,
                                                 o1t[:, no, qs])

                def emit_o2(do, qs, t):
                    o = outp.tile([P, QC], F32, tag="o", name=f"o{do}")
                    nc.vector.tensor_scalar(
                        o, t,
                        scalar1=g2_sb[:, do:do + 1],
                        scalar2=b2_sb[:, do:do + 1],
                        op0=ALU.mult, op1=ALU.add)
                    nc.sync.dma_start(ot3[:, do, qs], o)

                layernorm(z2t, sqp2, stp2, spp2, emit_o2)

    nc.compile()
    return nc


_CACHE = {}


def _get_nc():
    if "nc" not in _CACHE:
        _CACHE["nc"] = _build()
    return _CACHE["nc"]


def make_in_maps(X, Y, mask, Wq, bq, Wk, bk, Wv, bv, Wo, bo, g1, b1, g2, b2):
    f = lambda a: np.ascontiguousarray(np.asarray(a, dtype=np.float32))
    shared = {
        "Wq": f(Wq), "Wk": f(Wk), "Wv": f(Wv), "Wo": f(Wo),
        "bq": f(bq), "bk": f(bk), "bv": f(bv), "bo": f(bo),
        "g1": f(g1), "b1": f(b1), "g2": f(g2), "b2": f(b2),
    }
    X = np.asarray(X, dtype=np.float32)
    Y = np.asarray(Y, dtype=np.float32)
    mask = np.asarray(mask)
    in_maps = []
    for b in range(8):
        mb = np.where(mask[b], np.float32(-1e4), np.float32(0.0)).astype(np.float32)
        in_maps.append({
            "XT": np.ascontiguousarray(X[b].T),
            "YT": np.ascontiguousarray(Y[b].T),
            "MB": mb,
            **shared,
        })
    return in_maps


def kernel(X, Y, mask, Wq, bq, Wk, bk, Wv, bv, Wo, bo, g1, b1, g2, b2,
           _trace=False):
    nc = _get_nc()
    in_maps = make_in_maps(X, Y, mask, Wq, bq, Wk, bk, Wv, bv, Wo, bo,
                           g1, b1, g2, b2)
    res = run_bass_kernel_spmd(nc, in_maps, core_ids=list(range(8)),
                               trace=_trace)
    out = np.stack([np.ascontiguousarray(res.results[b]["OT"].T)
                    for b in range(8)]).astype(np.float32)
    if _trace:
        return out, res
    return out



# revision 4
# speedup vs baseline: 1.3655x; 1.3655x over previous
"""Trainium2 Bass kernel for a masked-attention block (MAB).

Computation (per batch element):
    Q = X@Wq + bq ; K = Y@Wk + bk ; V = Y@Wv + bv
    logits = per-head Qh@Kh^T / 32, masked keys -> -inf, softmax over keys
    attn   = A @ Vh (concat heads)
    O1 = LN(Q + attn; g1,b1)
    O  = LN(O1 + relu(O1@Wo + bo); g2,b2)

Sharding: pure data-parallel, one batch element per NeuronCore (B=8 = 8 cores).

On-device dataflow is "feature-major": activations live in SBUF transposed
([model_dim -> 8x128 partitions, token -> free]) so every matmul chains with
no transposes.  Softmax denominators and LayerNorm stats are partition-dim
reductions done with stationary-ones matmuls (which also broadcast the
result across partitions for free).

Precision: the Q path (residual-critical) is fp32r (FP22, full PE rate at
moving free >= 256); K/V/attention run bf16 (same PE rate, half the
DMA/SBUF traffic).  The key mask never touches the exp: masked keys are
excluded by zeroing V rows (per-partition ACT scale on the V-proj copy) and
by using a 0/1-mask-column matrix instead of all-ones in the denominator
reduction.  exp ACT ops therefore span two key-subtiles each.  The whole
kernel uses ONE ACT table (natural_log_exp_and_others: Exp/Ln/Identity/
Relu/Square): LayerNorm's rsigma is exp(-0.5*ln(var+eps)), so no mid-kernel
ACT table swaps ever happen.

Schedule: queries are processed in four 256-wide chunks pipelined through
attention -> LN1 -> O-proj -> LN2 so DVE/ACT LayerNorm work of one chunk
overlaps PE matmul work of the next and the PE never idles long enough for
HAM to re-throttle.  DMA emission order delivers K-proj operands first (PE
starts ~2us in), then Wq/X, then Wv, then the Wo prefetch; all weight/input
traffic is hidden behind phase-1 compute.
"""

import numpy as np
import ml_dtypes
from contextlib import ExitStack

import concourse.bass as bass
import concourse.mybir as mybir
import concourse.tile as tile
from concourse import bacc
from concourse.bass_utils import run_bass_kernel_spmd

P = 128
NX = 1024
NY = 1024
DIM = 1024
H = 8
KO = DIM // P          # 8 partition sub-tiles of the model dim
QC = 256               # query chunk
NQC = NX // QC         # 4
KC = 512               # key-side moving chunk for the projections
F32 = mybir.dt.float32
F32R = mybir.dt.float32r
BF16 = mybir.dt.bfloat16
AF = mybir.ActivationFunctionType
ALU = mybir.AluOpType
SCALE = 1.0 / 32.0     # 1/sqrt(DIM)
EPS = 1e-5
BF = ml_dtypes.bfloat16
# rsigma = exp(-0.5*ln(var+eps)) keeps everything in one ACT table; flip to
# the Sqrt + reciprocal path if the ln/exp tables ever lose too much accuracy
RSIG_VIA_LNEXP = True


def _r(ap):
    return ap.bitcast(F32R)


def _build():
    nc = bacc.Bacc("TRN2", target_bir_lowering=False, debug=False,
                   enable_asserts=False)

    # ---- DRAM I/O (per-core shapes) ----
    XT = nc.dram_tensor("XT", [DIM, NX], F32, kind="ExternalInput").ap()
    YTb = nc.dram_tensor("YTb", [DIM, NY], BF16, kind="ExternalInput").ap()
    MM01 = nc.dram_tensor("MM01", [P, KO, P], BF16, kind="ExternalInput").ap()
    M01 = nc.dram_tensor("M01", [P, KO], F32, kind="ExternalInput").ap()
    Wq = nc.dram_tensor("Wq", [DIM, DIM], F32, kind="ExternalInput").ap()
    Wkb = nc.dram_tensor("Wkb", [DIM, DIM], BF16, kind="ExternalInput").ap()
    Wvb = nc.dram_tensor("Wvb", [DIM, DIM], BF16, kind="ExternalInput").ap()
    Wo = nc.dram_tensor("Wo", [DIM, DIM], F32, kind="ExternalInput").ap()
    BVB = nc.dram_tensor("bvb", [NY], BF16, kind="ExternalInput").ap()
    Vecs = {}
    for vname in ("bq", "bk", "bo", "g1", "b1", "g2", "b2"):
        Vecs[vname] = nc.dram_tensor(vname, [DIM], F32, kind="ExternalInput").ap()
    OT = nc.dram_tensor("OT", [DIM, NX], F32, kind="ExternalOutput").ap()

    xt3 = XT.rearrange("(ko p) q -> p ko q", p=P)
    yt3 = YTb.rearrange("(ko p) q -> p ko q", p=P)
    wq3 = Wq.rearrange("(ko p) d -> p ko d", p=P)
    wk3 = Wkb.rearrange("(ko p) d -> p ko d", p=P)
    wv3 = Wvb.rearrange("(ko p) d -> p ko d", p=P)
    wo3 = Wo.rearrange("(ko p) d -> p ko d", p=P)
    ot3 = OT.rearrange("(do p) q -> p do q", p=P)

    with tile.TileContext(nc) as tc:
        with ExitStack() as octx:
            const = octx.enter_context(tc.tile_pool(name="const", bufs=1))

            # ---- constants ----
            ones128 = const.tile([P, P], F32, tag="ones", name="ones128")
            ones_tmp = const.tile([P, P], F32, tag="onest", name="ones_tmp")
            nc.vector.memset(ones_tmp, 1.0)
            nc.vector.tensor_copy(_r(ones128), ones_tmp)
            ones_bf = const.tile([P, P], BF16, tag="onesbf", name="ones_bf")
            nc.vector.memset(ones_bf, 1.0)
            eps_sb = const.tile([P, 1], F32, tag="eps", name="eps_sb")
            nc.vector.memset(eps_sb, EPS)

            def vec_pko(name):
                t = const.tile([P, KO], F32, tag=f"v_{name}", name=f"{name}_sb")
                nc.sync.dma_start(t, Vecs[name].rearrange("(ko p) -> p ko", p=P))
                return t

            bq_sb = vec_pko("bq")
            bk_sb = vec_pko("bk")
            bo_sb = vec_pko("bo")
            g1_sb = vec_pko("g1")
            b1_sb = vec_pko("b1")
            g2_sb = vec_pko("g2")
            b2_sb = vec_pko("b2")
            m01_sb = const.tile([P, KO], F32, tag="v_m01", name="m01_sb")
            nc.sync.dma_start(m01_sb, M01)
            mmat = const.tile([P, KO, P], BF16, tag="mmat", name="mmat")
            nc.sync.dma_start(mmat, MM01)
            bv_sb = const.tile([1, DIM], BF16, tag="v_bv", name="bv_sb")
            nc.sync.dma_start(bv_sb, BVB.rearrange("(one n) -> one n", one=1))

            # ---- long-lived activation tiles ----
            big = octx.enter_context(tc.tile_pool(name="big", bufs=1))
            ktm = big.tile([P, KO, NY], BF16, tag="ktm", name="ktm")
            vm = big.tile([P, KO, DIM], BF16, tag="vm", name="vm")
            qt = big.tile([P, KO, NX], F32, tag="qt", name="qt")
            qtb = big.tile([P, KO, NX], BF16, tag="qtb", name="qtb")

            # ============ Phase 1: K, Q, V projections ============
            # DMA emission = arrival order: ytb/wkb per-k pairs (K-proj
            # starts ~2us in, paced by DMA), then wq/xt, then wv, then Wo.
            with tc.tile_pool(name="io1", bufs=1) as iop, \
                 tc.tile_pool(name="xq", bufs=1) as xqp, \
                 tc.tile_pool(name="pp1", bufs=4, space="PSUM") as pp:
                ytb = iop.tile([P, KO, NY], BF16, tag="ytb", name="ytb")
                wkt = iop.tile([P, KO, DIM], BF16, tag="wkv", name="wkt")
                for k in range(KO):
                    nc.sync.dma_start(ytb[:, k, :], yt3[:, k, :])
                    nc.sync.dma_start(wkt[:, k, :], wk3[:, k, :])
                xt = xqp.tile([P, KO, NX], F32, tag="xt", name="xt")
                wqt = xqp.tile([P, KO, DIM], F32, tag="wqt", name="wqt")
                for k in range(KO):
                    nc.sync.dma_start(_r(wqt[:, k, :]), _r(wq3[:, k, :]))
                for c in range(NQC):
                    qs = slice(c * QC, (c + 1) * QC)
                    for k in range(KO):
                        nc.sync.dma_start(_r(xt[:, k, qs]), _r(xt3[:, k, qs]))

                # ---- K-proj (bf16): ktm[p,do,key] = sum_k Wk[k,d] Y^T[k,key]
                for grp in range(2):
                    pss = [pp.tile([P, 2, KC], F32, tag="ps",
                                   name=f"ps_k{grp}{i}") for i in range(4)]
                    for k in range(KO):
                        for i in range(4):
                            do = grp * 4 + i
                            for ng in range(2):
                                nc.tensor.matmul(
                                    pss[i][:, ng, :],
                                    lhsT=wkt[:, k, do * P:(do + 1) * P],
                                    rhs=ytb[:, k, ng * KC:(ng + 1) * KC],
                                    start=(k == 0), stop=(k == KO - 1))
                    for i in range(4):
                        do = grp * 4 + i
                        nc.scalar.activation(
                            ktm[:, do, :], pss[i], AF.Identity,
                            bias=bk_sb[:, do:do + 1], scale=1.0)

                # ---- Q-proj (fp32r), chunked; dual-store f32 + bf16
                for c in range(NQC):
                    qs = slice(c * QC, (c + 1) * QC)
                    for do in range(KO):
                        ps = pp.tile([P, 2, KC], F32, tag="ps",
                                     name=f"ps_q{c}{do}")
                        pq = ps[:, 0, 0:QC]
                        for k in range(KO):
                            nc.tensor.matmul(
                                pq,
                                lhsT=_r(wqt[:, k, do * P:(do + 1) * P]),
                                rhs=_r(xt[:, k, qs]),
                                start=(k == 0), stop=(k == KO - 1))
                        nc.scalar.activation(
                            _r(qt[:, do, qs]), pq, AF.Identity,
                            bias=bq_sb[:, do:do + 1], scale=1.0)
                        nc.scalar.activation(
                            qtb[:, do, qs], pq, AF.Identity,
                            bias=bq_sb[:, do:do + 1], scale=1.0)

                # ---- V-proj (bf16, natural layout, mask-zeroed rows)
                wvt = iop.tile([P, KO, DIM], BF16, tag="wkv", name="wvt")
                for k in range(KO):
                    nc.sync.dma_start(wvt[:, k, :], wv3[:, k, :])
                for yo in range(KO):
                    ps = pp.tile([P, 2, KC], F32, tag="ps", name=f"ps_v{yo}")
                    for k in range(KO):
                        for ng in range(2):
                            nc.tensor.matmul(
                                ps[:, ng, :],
                                lhsT=ytb[:, k, yo * P:(yo + 1) * P],
                                rhs=wvt[:, k, ng * KC:(ng + 1) * KC],
                                start=(k == 0), stop=False)
                    for ng in range(2):
                        nc.tensor.matmul(
                            ps[:, ng, :], lhsT=ones_bf[0:1, :],
                            rhs=bv_sb[:, ng * KC:(ng + 1) * KC],
                            start=False, stop=True)
                    # 0/1 mask per key row -> masked V rows become 0
                    nc.scalar.activation(
                        vm[:, yo, :], ps, AF.Identity,
                        scale=m01_sb[:, yo:yo + 1])

            # ============ Phase 2+3: per-query-chunk pipeline ============
            wop = octx.enter_context(tc.tile_pool(name="wop", bufs=1))
            wot = wop.tile([P, KO, DIM], F32, tag="wot", name="wot")
            for k in range(KO):
                nc.sync.dma_start(_r(wot[:, k, :]), _r(wo3[:, k, :]))

            stg = octx.enter_context(tc.tile_pool(name="stg", bufs=1))
            ep = octx.enter_context(tc.tile_pool(name="exp", bufs=2))
            rp = octx.enter_context(tc.tile_pool(name="rcp", bufs=2))
            sqp = octx.enter_context(tc.tile_pool(name="sq", bufs=1))
            stp = octx.enter_context(tc.tile_pool(name="st", bufs=6))
            outp = octx.enter_context(tc.tile_pool(name="out", bufs=4))
            lgp = octx.enter_context(tc.tile_pool(name="lgp", bufs=2, space="PSUM"))
            avp = octx.enter_context(tc.tile_pool(name="avp", bufs=2, space="PSUM"))
            rlp = octx.enter_context(tc.tile_pool(name="rlp", bufs=2, space="PSUM"))
            spp = octx.enter_context(tc.tile_pool(name="spp", bufs=2, space="PSUM"))

            # stage buffers: zt/z2t share rotation slots; o1t has its own
            zts = [stg.tile([P, KO, QC], F32, tag="zz", bufs=3, name=f"zt{c}")
                   for c in range(NQC)]
            z2ts = [stg.tile([P, KO, QC], F32, tag="zz", bufs=3, name=f"z2t{c}")
                    for c in range(NQC)]
            o1ts = [stg.tile([P, KO, QC], F32, tag="o1", bufs=2, name=f"o1t{c}")
                    for c in range(NQC)]

            def logits_exp(c, h):
                # logitsT[key, q] = sum_d K^T_h[d, key] Q^T_h[d, q]; exp via
                # ACT over 2 key-subtiles at once (mask not applied here).
                qs = slice(c * QC, (c + 1) * QC)
                et = ep.tile([P, KO, QC], BF16, tag="exp", name=f"et{c}_{h}")
                for kp in range(4):
                    pl = lgp.tile([P, 2, QC], F32, tag="lg", name=f"pl{c}{h}{kp}")
                    for j in range(2):
                        kt = 2 * kp + j
                        nc.tensor.matmul(
                            pl[:, j, :],
                            lhsT=ktm[:, h, kt * P:(kt + 1) * P],
                            rhs=qtb[:, h, qs],
                            start=True, stop=True)
                    nc.scalar.activation(
                        et[:, 2 * kp:2 * kp + 2, :], pl, AF.Exp, scale=SCALE)
                return et

            def denom_av(c, h, et):
                qs = slice(c * QC, (c + 1) * QC)
                # denominator: mask-column matrix instead of all-ones both
                # excludes masked keys and reduces/broadcasts across
                # partitions in one accumulation chain.
                pr = rlp.tile([P, QC], F32, tag="rl", name=f"pr{c}{h}")
                for kt in range(KO):
                    nc.tensor.matmul(
                        pr, lhsT=mmat[:, kt, :], rhs=et[:, kt, :],
                        start=(kt == 0), stop=(kt == KO - 1))
                rc = rp.tile([P, QC], F32, tag="rc", name=f"rc{c}{h}")
                nc.vector.reciprocal_approx_fast(rc, pr)
                # attnT_h[d, q] = sum_key V[key, d_h] expT[key, q] (masked V
                # rows are zero); normalize and add the Q residual.
                pa = avp.tile([P, QC], F32, tag="av", name=f"pa{c}{h}")
                for kt in range(KO):
                    nc.tensor.matmul(
                        pa,
                        lhsT=vm[:, kt, h * P:(h + 1) * P],
                        rhs=et[:, kt, :],
                        start=(kt == 0), stop=(kt == KO - 1))
                nc.vector.tensor_mul(_r(zts[c][:, h, :]), pa, rc)
                nc.vector.tensor_add(_r(zts[c][:, h, :]), zts[c][:, h, :],
                                     qt[:, h, qs])

            def attn_chunk(c):
                prev = None
                for h in range(H):
                    et = logits_exp(c, h)
                    if prev is not None:
                        denom_av(c, h - 1, prev)
                    prev = et
                denom_av(c, H - 1, prev)

            def layernorm(c, tag, in_sb, g_sb, b_sb, emit_out):
                # stats: partition-dim mean/meansq via stationary-ones MMs;
                # squares on ACT; normalize on DVE; g/b fold on ACT.
                pmu = spp.tile([P, QC], F32, tag="sp", name=f"pmu{tag}{c}")
                ps2 = spp.tile([P, QC], F32, tag="sp", name=f"ps2{tag}{c}")
                for do in range(KO):
                    nc.tensor.matmul(pmu, lhsT=_r(ones128),
                                     rhs=_r(in_sb[:, do, :]),
                                     start=(do == 0), stop=(do == KO - 1))
                sqs = []
                for do in range(KO):
                    sq = sqp.tile([P, QC], F32, tag="sq", bufs=8,
                                  name=f"sq{tag}{c}{do}")
                    nc.scalar.activation(_r(sq), in_sb[:, do, :], AF.Square)
                    sqs.append(sq)
                for do in range(KO):
                    nc.tensor.matmul(ps2, lhsT=_r(ones128), rhs=_r(sqs[do]),
                                     start=(do == 0), stop=(do == KO - 1))
                mu = stp.tile([P, QC], F32, tag="st", name=f"mu{tag}{c}")
                nc.vector.tensor_scalar_mul(mu, pmu, 1.0 / DIM)
                msq = stp.tile([P, QC], F32, tag="st", name=f"msq{tag}{c}")
                nc.vector.tensor_mul(msq, mu, mu)
                var = stp.tile([P, QC], F32, tag="st", name=f"var{tag}{c}")
                nc.vector.scalar_tensor_tensor(
                    var, ps2, 1.0 / DIM, msq,
                    op0=ALU.mult, op1=ALU.subtract)
                rsig = stp.tile([P, QC], F32, tag="st", name=f"rsig{tag}{c}")
                if RSIG_VIA_LNEXP:
                    lnv = stp.tile([P, QC], F32, tag="st", name=f"lnv{tag}{c}")
                    nc.scalar.activation(lnv, var, AF.Ln, bias=eps_sb, scale=1.0)
                    nc.scalar.activation(rsig, lnv, AF.Exp, scale=-0.5)
                else:
                    sd = stp.tile([P, QC], F32, tag="st", name=f"sd{tag}{c}")
                    nc.scalar.activation(sd, var, AF.Sqrt, bias=eps_sb, scale=1.0)
                    nc.vector.reciprocal_approx_fast(rsig, sd)
                for do in range(KO):
                    t = sqp.tile([P, QC], F32, tag="t", bufs=3,
                                 name=f"t{tag}{c}{do}")
                    nc.vector.tensor_sub(t, in_sb[:, do, :], mu)
                    nc.vector.tensor_mul(t, t, rsig)
                    emit_out(do, t, g_sb, b_sb)

            def ln1_chunk(c):
                def emit_o1(do, t, g_sb, b_sb):
                    nc.scalar.activation(
                        _r(o1ts[c][:, do, :]), t, AF.Identity,
                        bias=b_sb[:, do:do + 1], scale=g_sb[:, do:do + 1])
                layernorm(c, "a", zts[c], g1_sb, b1_sb, emit_o1)

            def oproj_chunk(c):
                # H^T[n, q] = sum_d Wo[d, n] O1^T[d, q]; z2 = o1 + relu(H+bo)
                for no in range(KO):
                    ps = avp.tile([P, QC], F32, tag="av", name=f"ps_o{c}{no}")
                    for k in range(KO):
                        nc.tensor.matmul(
                            ps,
                            lhsT=_r(wot[:, k, no * P:(no + 1) * P]),
                            rhs=_r(o1ts[c][:, k, :]),
                            start=(k == 0), stop=(k == KO - 1))
                    ht = sqp.tile([P, QC], F32, tag="ht", bufs=3,
                                  name=f"ht{c}{no}")
                    nc.scalar.activation(ht, ps, AF.Relu,
                                         bias=bo_sb[:, no:no + 1], scale=1.0)
                    nc.vector.tensor_add(_r(z2ts[c][:, no, :]), ht,
                                         o1ts[c][:, no, :])

            def ln2_chunk(c):
                qs = slice(c * QC, (c + 1) * QC)

                def emit_o2(do, t, g_sb, b_sb):
                    o = outp.tile([P, QC], F32, tag="o", name=f"o{c}{do}")
                    nc.scalar.activation(
                        o, t, AF.Identity,
                        bias=b_sb[:, do:do + 1], scale=g_sb[:, do:do + 1])
                    nc.sync.dma_start(ot3[:, do, qs], o)
                layernorm(c, "b", z2ts[c], g2_sb, b2_sb, emit_o2)

            for c in range(NQC):
                attn_chunk(c)
                ln1_chunk(c)
            for c in range(NQC):
                oproj_chunk(c)
                ln2_chunk(c)

    nc.compile()
    return nc


_CACHE = {}


def _get_nc():
    if "nc" not in _CACHE:
        _CACHE["nc"] = _build()
    return _CACHE["nc"]


def make_in_maps(X, Y, mask, Wq, bq, Wk, bk, Wv, bv, Wo, bo, g1, b1, g2, b2):
    f = lambda a: np.ascontiguousarray(np.asarray(a, dtype=np.float32))
    fb = lambda a: np.ascontiguousarray(np.asarray(a, dtype=np.float32).astype(BF))
    shared = {
        "Wq": f(Wq), "Wkb": fb(Wk), "Wvb": fb(Wv), "Wo": f(Wo),
        "bvb": fb(bv),
        "bq": f(bq), "bk": f(bk), "bo": f(bo),
        "g1": f(g1), "b1": f(b1), "g2": f(g2), "b2": f(b2),
    }
    X = np.asarray(X, dtype=np.float32)
    Y = np.asarray(Y, dtype=np.float32)
    mask = np.asarray(mask)
    in_maps = []
    for b in range(8):
        m01 = np.where(mask[b], np.float32(0.0), np.float32(1.0))
        m01_pk = np.ascontiguousarray(m01.reshape(KO, P).T)      # [P, KO]
        mm01 = np.ascontiguousarray(
            np.broadcast_to(m01_pk[:, :, None], (P, KO, P))).astype(BF)
        in_maps.append({
            "XT": np.ascontiguousarray(X[b].T),
            "YTb": np.ascontiguousarray(Y[b].T.astype(BF)),
            "M01": m01_pk,
            "MM01": mm01,
            **shared,
        })
    return in_maps


def kernel(X, Y, mask, Wq, bq, Wk, bk, Wv, bv, Wo, bo, g1, b1, g2, b2,
           _trace=False):
    nc = _get_nc()
    in_maps = make_in_maps(X, Y, mask, Wq, bq, Wk, bk, Wv, bv, Wo, bo,
                           g1, b1, g2, b2)
    res = run_bass_kernel_spmd(nc, in_maps, core_ids=list(range(8)),
                               trace=_trace)
    out = np.stack([np.ascontiguousarray(res.results[b]["OT"].T)
                    for b in range(8)]).astype(np.float32)
    if _trace:
        return out, res
    return out


# revision 6
# speedup vs baseline: 1.4141x; 1.0356x over previous
"""Trainium2 Bass kernel for a masked-attention block (MAB).

Computation (per batch element):
    Q = X@Wq + bq ; K = Y@Wk + bk ; V = Y@Wv + bv
    logits = per-head Qh@Kh^T / 32, masked keys -> -inf, softmax over keys
    attn   = A @ Vh (concat heads)
    O1 = LN(Q + attn; g1,b1)
    O  = LN(O1 + relu(O1@Wo + bo); g2,b2)

Sharding: pure data-parallel, one batch element per NeuronCore (B=8 = 8 cores).

On-device dataflow is "feature-major": activations live in SBUF transposed
([model_dim -> 8x128 partitions, token -> free]) so every matmul chains with
no transposes.  Softmax denominators and LayerNorm stats are partition-dim
reductions done with stationary-ones matmuls (which also broadcast the
result across partitions for free).  The key mask folds into the exp as a
per-partition additive bias (0 / -1e4).

Everything runs bf16 into fp32 PSUM (bf16 matmuls stream at the same
1 cycle/row as fp32r but enable FWL weight loads, halve DMA bytes, and give
the DVE its 2-4x 16-bit modes).  The whole kernel uses ONE ACT table
(natural_log_exp_and_others): LayerNorm's rsigma is exp(-0.5*ln(var+eps)),
and the table list is reordered so Exp and Ln resolve to that shared table
-- zero mid-kernel ACT table swaps.

Schedule: queries run in two 512-wide chunks pipelined through attention ->
LN1 -> O-proj -> LN2, with denominator/AV matmuls of head h-1 interleaved
into the logits matmuls of head h, so the PE stream stays dense and HAM
never re-throttles.  The packed-constant DMA plus bf16 inputs put the first
matmul ~5us in; Wv/Wo prefetch behind the phase-1 operand stream.
"""

import numpy as np
import ml_dtypes
from contextlib import ExitStack

import concourse.bass as bass
import concourse.mybir as mybir
import concourse.tile as tile
from concourse import bacc
from concourse.bass_utils import run_bass_kernel_spmd

P = 128
NX = 1024
NY = 1024
DIM = 1024
H = 8
KO = DIM // P          # 8 partition sub-tiles of the model dim
QC = 512               # query chunk
NQC = NX // QC         # 2
F32 = mybir.dt.float32
BF16 = mybir.dt.bfloat16
AF = mybir.ActivationFunctionType
ALU = mybir.AluOpType
SCALE = 1.0 / 32.0     # 1/sqrt(DIM)
EPS = 1e-5
BF = ml_dtypes.bfloat16
# packed per-partition vectors: [P, KO, NVEC]
VNAMES = ("bq", "bk", "bo", "g1", "b1", "g2", "b2", "mb")
NVEC = len(VNAMES)

_TBL_PATCHED = False


def _patch_act_tables():
    """Steer every activation in this kernel to the ONE table that contains
    all of Exp/Ln/Identity/Relu/Copy (natural_log_exp_and_others) -> a single
    table load, no mid-kernel swaps.  act_func_set_id is positional into
    act_info.json, so the list ORDER must not change; instead strip this
    kernel's functions from every other table's set so the chooser picks the
    combined table at its true index."""
    global _TBL_PATCHED
    if _TBL_PATCHED:
        return
    orig = bacc.get_activation_tables

    def steered(arch):
        tabs = orig(arch)
        pref = "natural_log_exp_and_others"
        mine = {AF.Exp, AF.Ln, AF.Identity, AF.Relu, AF.Copy}
        if pref in tabs and mine <= set(tabs[pref]):
            return {k: (v if k == pref else set(v) - mine)
                    for k, v in tabs.items()}
        return tabs

    bacc.get_activation_tables = steered
    _TBL_PATCHED = True


def _build():
    _patch_act_tables()
    nc = bacc.Bacc("TRN2", target_bir_lowering=False, debug=False,
                   enable_asserts=False)

    # ---- DRAM I/O (per-core shapes) ----
    XTb = nc.dram_tensor("XTb", [DIM, NX], BF16, kind="ExternalInput").ap()
    YTb = nc.dram_tensor("YTb", [DIM, NY], BF16, kind="ExternalInput").ap()
    VPK = nc.dram_tensor("VPK", [P, KO, NVEC], F32, kind="ExternalInput").ap()
    Wqb = nc.dram_tensor("Wqb", [DIM, DIM], BF16, kind="ExternalInput").ap()
    Wkb = nc.dram_tensor("Wkb", [DIM, DIM], BF16, kind="ExternalInput").ap()
    Wvb = nc.dram_tensor("Wvb", [DIM, DIM], BF16, kind="ExternalInput").ap()
    Wob = nc.dram_tensor("Wob", [DIM, DIM], BF16, kind="ExternalInput").ap()
    BVB = nc.dram_tensor("bvb", [NY], BF16, kind="ExternalInput").ap()
    OT = nc.dram_tensor("OT", [DIM, NX], F32, kind="ExternalOutput").ap()

    xt3 = XTb.rearrange("(ko p) q -> p ko q", p=P)
    yt3 = YTb.rearrange("(ko p) q -> p ko q", p=P)
    wq3 = Wqb.rearrange("(ko p) d -> p ko d", p=P)
    wk3 = Wkb.rearrange("(ko p) d -> p ko d", p=P)
    wv3 = Wvb.rearrange("(ko p) d -> p ko d", p=P)
    wo3 = Wob.rearrange("(ko p) d -> p ko d", p=P)
    ot3 = OT.rearrange("(do p) q -> p do q", p=P)

    with tile.TileContext(nc) as tc:
        with ExitStack() as octx:
            const = octx.enter_context(tc.tile_pool(name="const", bufs=1))

            # ---- constants (packed: 2 fast DMAs ahead of the big streams)
            vpk = const.tile([P, KO, NVEC], F32, tag="vpk", name="vpk")
            nc.sync.dma_start(vpk, VPK)
            bv_sb = const.tile([1, DIM], BF16, tag="v_bv", name="bv_sb")
            nc.sync.dma_start(bv_sb, BVB.rearrange("(one n) -> one n", one=1))
            V = {name: i for i, name in enumerate(VNAMES)}

            def vec(name, do):
                i = V[name]
                return vpk[:, do, i:i + 1]

            ones_bf = const.tile([P, P], BF16, tag="onesbf", name="ones_bf")
            nc.vector.memset(ones_bf, 1.0)
            eps_sb = const.tile([P, 1], F32, tag="eps", name="eps_sb")
            nc.vector.memset(eps_sb, EPS)

            # ---- long-lived activation tiles ----
            big = octx.enter_context(tc.tile_pool(name="big", bufs=1))
            ktm = big.tile([P, KO, NY], BF16, tag="ktm", name="ktm")
            vm = big.tile([P, KO, DIM], BF16, tag="vm", name="vm")
            qtb = big.tile([P, KO, NX], BF16, tag="qtb", name="qtb")

            # ============ Phase 1: K, Q, V projections ============
            # DMA emission = arrival order: ytb/wkb per-k pairs (K-proj
            # starts ~4us in, paced by DMA), then xtb/wqb, then wvb, wob.
            with tc.tile_pool(name="io1", bufs=1) as iop, \
                 tc.tile_pool(name="xq", bufs=1) as xqp, \
                 tc.tile_pool(name="pp1", bufs=4, space="PSUM") as pp:
                ytb = iop.tile([P, KO, NY], BF16, tag="ytb", name="ytb")
                wkt = iop.tile([P, KO, DIM], BF16, tag="wkv", name="wkt")
                for k in range(KO):
                    nc.sync.dma_start(ytb[:, k, :], yt3[:, k, :])
                    nc.sync.dma_start(wkt[:, k, :], wk3[:, k, :])
                xtb = xqp.tile([P, KO, NX], BF16, tag="xtb", name="xtb")
                wqt = xqp.tile([P, KO, DIM], BF16, tag="wqt", name="wqt")
                for k in range(KO):
                    nc.sync.dma_start(xtb[:, k, :], xt3[:, k, :])
                    nc.sync.dma_start(wqt[:, k, :], wq3[:, k, :])

                # ---- K-proj: ktm[p,do,key] = sum_k Wk[k,d] Y^T[k,key]
                for grp in range(2):
                    pss = [pp.tile([P, 2, QC], F32, tag="ps",
                                   name=f"ps_k{grp}{i}") for i in range(4)]
                    for k in range(KO):
                        for i in range(4):
                            do = grp * 4 + i
                            for ng in range(2):
                                nc.tensor.matmul(
                                    pss[i][:, ng, :],
                                    lhsT=wkt[:, k, do * P:(do + 1) * P],
                                    rhs=ytb[:, k, ng * QC:(ng + 1) * QC],
                                    start=(k == 0), stop=(k == KO - 1))
                    for i in range(4):
                        do = grp * 4 + i
                        nc.scalar.activation(
                            ktm[:, do, :], pss[i], AF.Identity,
                            bias=vec("bk", do), scale=1.0)

                # ---- Q-proj, chunked; bf16 store (residual uses qtb too)
                for c in range(NQC):
                    qs = slice(c * QC, (c + 1) * QC)
                    for do in range(KO):
                        ps = pp.tile([P, 2, QC], F32, tag="ps",
                                     name=f"ps_q{c}{do}")
                        pq = ps[:, 0, :]
                        for k in range(KO):
                            nc.tensor.matmul(
                                pq,
                                lhsT=wqt[:, k, do * P:(do + 1) * P],
                                rhs=xtb[:, k, qs],
                                start=(k == 0), stop=(k == KO - 1))
                        nc.scalar.activation(
                            qtb[:, do, qs], pq, AF.Identity,
                            bias=vec("bq", do), scale=1.0)

                # ---- V-proj (natural layout; per-free bias via K=1 MM)
                wvt = iop.tile([P, KO, DIM], BF16, tag="wkv", name="wvt")
                for k in range(KO):
                    nc.sync.dma_start(wvt[:, k, :], wv3[:, k, :])
                for yo in range(KO):
                    ps = pp.tile([P, 2, QC], F32, tag="ps", name=f"ps_v{yo}")
                    for k in range(KO):
                        for ng in range(2):
                            nc.tensor.matmul(
                                ps[:, ng, :],
                                lhsT=ytb[:, k, yo * P:(yo + 1) * P],
                                rhs=wvt[:, k, ng * QC:(ng + 1) * QC],
                                start=(k == 0), stop=False)
                    for ng in range(2):
                        nc.tensor.matmul(
                            ps[:, ng, :], lhsT=ones_bf[0:1, :],
                            rhs=bv_sb[:, ng * QC:(ng + 1) * QC],
                            start=False, stop=True)
                    nc.scalar.copy(vm[:, yo, :], ps)

            # ============ Phase 2+3: per-query-chunk pipeline ============
            wop = octx.enter_context(tc.tile_pool(name="wop", bufs=1))
            wot = wop.tile([P, KO, DIM], BF16, tag="wot", name="wot")
            for k in range(KO):
                nc.sync.dma_start(wot[:, k, :], wo3[:, k, :])

            stg = octx.enter_context(tc.tile_pool(name="stg", bufs=1))
            ep = octx.enter_context(tc.tile_pool(name="exp", bufs=3))
            rp = octx.enter_context(tc.tile_pool(name="rcp", bufs=2))
            sqp = octx.enter_context(tc.tile_pool(name="sq", bufs=1))
            stp = octx.enter_context(tc.tile_pool(name="st", bufs=8))
            outp = octx.enter_context(tc.tile_pool(name="out", bufs=4))
            lgp = octx.enter_context(tc.tile_pool(name="lgp", bufs=4, space="PSUM"))
            avp = octx.enter_context(tc.tile_pool(name="avp", bufs=2, space="PSUM"))
            rlp = octx.enter_context(tc.tile_pool(name="rlp", bufs=2, space="PSUM"))

            zts = [stg.tile([P, KO, QC], BF16, tag="zz", bufs=3, name=f"zt{c}")
                   for c in range(NQC)]
            z2ts = [stg.tile([P, KO, QC], BF16, tag="zz", bufs=3, name=f"z2t{c}")
                    for c in range(NQC)]
            o1ts = [stg.tile([P, KO, QC], BF16, tag="o1", bufs=2, name=f"o1t{c}")
                    for c in range(NQC)]

            def attn_chunk(c):
                # per head: 8 logits MMs -> per-kt exp (mask bias) -> 8
                # denominator ones-MMs + 8 AV MMs; the denominator/AV of
                # head h-1 is interleaved into the logits of head h so the
                # PE stream stays dense while ACT drains the exps.
                qs = slice(c * QC, (c + 1) * QC)
                state = {}

                def logits_part(h, kts):
                    et = state.setdefault(
                        h, ep.tile([P, KO, QC], BF16, tag="exp",
                                   name=f"et{c}_{h}"))
                    for kt in kts:
                        pl = lgp.tile([P, QC], F32, tag="lg",
                                      name=f"pl{c}{h}{kt}")
                        nc.tensor.matmul(
                            pl, lhsT=ktm[:, h, kt * P:(kt + 1) * P],
                            rhs=qtb[:, h, qs], start=True, stop=True)
                        nc.scalar.activation(
                            et[:, kt, :], pl, AF.Exp,
                            bias=vec("mb", kt), scale=SCALE)

                def denom_part(h, kts):
                    et = state[h]
                    pr = state.setdefault(
                        ("pr", h), rlp.tile([P, QC], F32, tag="rl",
                                            name=f"pr{c}{h}"))
                    for kt in kts:
                        nc.tensor.matmul(
                            pr, lhsT=ones_bf, rhs=et[:, kt, :],
                            start=(kt == 0), stop=(kt == KO - 1))
                    if kts[-1] == KO - 1:
                        rc = rp.tile([P, QC], F32, tag="rc", name=f"rc{c}{h}")
                        state[("rc", h)] = rc
                        nc.vector.reciprocal_approx_fast(rc, pr)

                def av_part(h, kts):
                    et = state[h]
                    pa = state.setdefault(
                        ("pa", h), avp.tile([P, QC], F32, tag="av",
                                            name=f"pa{c}{h}"))
                    for kt in kts:
                        nc.tensor.matmul(
                            pa, lhsT=vm[:, kt, h * P:(h + 1) * P],
                            rhs=et[:, kt, :],
                            start=(kt == 0), stop=(kt == KO - 1))
                    if kts[-1] == KO - 1:
                        rc = state[("rc", h)]
                        nc.vector.tensor_mul(zts[c][:, h, :], pa, rc)
                        nc.vector.tensor_add(zts[c][:, h, :], zts[c][:, h, :],
                                             qtb[:, h, qs])

                for h in range(H):
                    for step in range(4):
                        logits_part(h, [2 * step, 2 * step + 1])
                        if h > 0:
                            if step < 2:
                                denom_part(h - 1, [4 * step + j
                                                   for j in range(4)])
                            else:
                                av_part(h - 1, [4 * (step - 2) + j
                                                for j in range(4)])
                denom_part(H - 1, list(range(4)))
                denom_part(H - 1, list(range(4, KO)))
                av_part(H - 1, list(range(4)))
                av_part(H - 1, list(range(4, KO)))

            def layernorm(c, tag, in_sb, gname, bname, emit_out):
                # stats: partition-dim mean/meansq via stationary-ones MMs
                # (bf16 input, fp32 psum); squares on DVE (4x bf16 mode);
                # rsigma = exp(-0.5*ln(var+eps)) on ACT (same table as the
                # softmax exp); normalize on DVE in bf16.
                pmu = rlp.tile([P, QC], F32, tag="rl", name=f"pmu{tag}{c}")
                ps2 = rlp.tile([P, QC], F32, tag="rl", name=f"ps2{tag}{c}")
                for do in range(KO):
                    nc.tensor.matmul(pmu, lhsT=ones_bf,
                                     rhs=in_sb[:, do, :],
                                     start=(do == 0), stop=(do == KO - 1))
                sqs = []
                for do in range(KO):
                    sq = sqp.tile([P, QC], BF16, tag="sq", bufs=8,
                                  name=f"sq{tag}{c}{do}")
                    nc.vector.tensor_mul(sq, in_sb[:, do, :], in_sb[:, do, :])
                    sqs.append(sq)
                for do in range(KO):
                    nc.tensor.matmul(ps2, lhsT=ones_bf, rhs=sqs[do],
                                     start=(do == 0), stop=(do == KO - 1))
                mu = stp.tile([P, QC], F32, tag="st", name=f"mu{tag}{c}")
                nc.vector.tensor_scalar_mul(mu, pmu, 1.0 / DIM)
                msq = stp.tile([P, QC], F32, tag="st", name=f"msq{tag}{c}")
                nc.vector.tensor_mul(msq, mu, mu)
                var = stp.tile([P, QC], F32, tag="st", name=f"var{tag}{c}")
                nc.vector.scalar_tensor_tensor(
                    var, ps2, 1.0 / DIM, msq,
                    op0=ALU.mult, op1=ALU.subtract)
                lnv = stp.tile([P, QC], F32, tag="st", name=f"lnv{tag}{c}")
                nc.scalar.activation(lnv, var, AF.Ln, bias=eps_sb, scale=1.0)
                rsig = stp.tile([P, QC], F32, tag="st", name=f"rsig{tag}{c}")
                nc.scalar.activation(rsig, lnv, AF.Exp, scale=-0.5)
                mub = stp.tile([P, QC], BF16, tag="stb", bufs=2,
                               name=f"mub{tag}{c}")
                nc.vector.tensor_copy(mub, mu)
                rsb = stp.tile([P, QC], BF16, tag="stb", bufs=2,
                               name=f"rsb{tag}{c}")
                nc.vector.tensor_copy(rsb, rsig)
                for do in range(KO):
                    t = sqp.tile([P, QC], BF16, tag="t", bufs=3,
                                 name=f"t{tag}{c}{do}")
                    nc.vector.tensor_sub(t, in_sb[:, do, :], mub)
                    nc.vector.tensor_mul(t, t, rsb)
                    emit_out(do, t)

            def ln1_chunk(c):
                def emit_o1(do, t):
                    nc.vector.tensor_scalar(
                        o1ts[c][:, do, :], t,
                        scalar1=vec("g1", do), scalar2=vec("b1", do),
                        op0=ALU.mult, op1=ALU.add)
                layernorm(c, "a", zts[c], "g1", "b1", emit_o1)

            def oproj_chunk(c):
                # H^T[n, q] = sum_d Wo[d, n] O1^T[d, q]; z2 = o1 + relu(H+bo)
                for no in range(KO):
                    ps = avp.tile([P, QC], F32, tag="av", name=f"ps_o{c}{no}")
                    for k in range(KO):
                        nc.tensor.matmul(
                            ps,
                            lhsT=wot[:, k, no * P:(no + 1) * P],
                            rhs=o1ts[c][:, k, :],
                            start=(k == 0), stop=(k == KO - 1))
                    ht = sqp.tile([P, QC], BF16, tag="ht", bufs=3,
                                  name=f"ht{c}{no}")
                    nc.scalar.activation(ht, ps, AF.Relu,
                                         bias=vec("bo", no), scale=1.0)
                    nc.vector.tensor_add(z2ts[c][:, no, :], ht,
                                         o1ts[c][:, no, :])

            def ln2_chunk(c):
                qs = slice(c * QC, (c + 1) * QC)

                def emit_o2(do, t):
                    o = outp.tile([P, QC], F32, tag="o", name=f"o{c}{do}")
                    nc.scalar.activation(
                        o, t, AF.Identity,
                        bias=vec("b2", do), scale=vec("g2", do))
                    nc.sync.dma_start(ot3[:, do, qs], o)
                layernorm(c, "b", z2ts[c], "g2", "b2", emit_o2)

            attn_chunk(0)
            ln1_chunk(0)
            attn_chunk(1)
            ln1_chunk(1)
            oproj_chunk(0)
            ln2_chunk(0)
            oproj_chunk(1)
            ln2_chunk(1)

    nc.compile()
    return nc


_CACHE = {}


def _get_nc():
    if "nc" not in _CACHE:
        _CACHE["nc"] = _build()
    return _CACHE["nc"]


def make_in_maps(X, Y, mask, Wq, bq, Wk, bk, Wv, bv, Wo, bo, g1, b1, g2, b2):
    fb = lambda a: np.ascontiguousarray(np.asarray(a, dtype=np.float32).astype(BF))
    shared = {
        "Wqb": fb(Wq), "Wkb": fb(Wk), "Wvb": fb(Wv), "Wob": fb(Wo),
        "bvb": fb(bv),
    }
    svecs = {
        "bq": np.asarray(bq, np.float32), "bk": np.asarray(bk, np.float32),
        "bo": np.asarray(bo, np.float32), "g1": np.asarray(g1, np.float32),
        "b1": np.asarray(b1, np.float32), "g2": np.asarray(g2, np.float32),
        "b2": np.asarray(b2, np.float32),
    }
    X = np.asarray(X, dtype=np.float32)
    Y = np.asarray(Y, dtype=np.float32)
    mask = np.asarray(mask)
    in_maps = []
    for b in range(8):
        mb = np.where(mask[b], np.float32(-1e4), np.float32(0.0))
        vpk = np.zeros((P, KO, NVEC), np.float32)
        for i, name in enumerate(VNAMES):
            v = mb if name == "mb" else svecs[name]
            vpk[:, :, i] = v.reshape(KO, P).T
        in_maps.append({
            "XTb": np.ascontiguousarray(X[b].T.astype(BF)),
            "YTb": np.ascontiguousarray(Y[b].T.astype(BF)),
            "VPK": np.ascontiguousarray(vpk),
            **shared,
        })
    return in_maps


def kernel(X, Y, mask, Wq, bq, Wk, bk, Wv, bv, Wo, bo, g1, b1, g2, b2,
           _trace=False):
    nc = _get_nc()
    in_maps = make_in_maps(X, Y, mask, Wq, bq, Wk, bk, Wv, bv, Wo, bo,
                           g1, b1, g2, b2)
    res = run_bass_kernel_spmd(nc, in_maps, core_ids=list(range(8)),
                               trace=_trace)
    out = np.stack([np.ascontiguousarray(res.results[b]["OT"].T)
                    for b in range(8)]).astype(np.float32)
    if _trace:
        return out, res
    return out


# revision 14
# speedup vs baseline: 1.7148x; 1.2126x over previous
"""Trainium2 Bass kernel for a masked-attention block (MAB).

Computation (per batch element):
    Q = X@Wq + bq ; K = Y@Wk + bk ; V = Y@Wv + bv
    logits = per-head Qh@Kh^T / 32, masked keys -> -inf, softmax over keys
    attn   = A @ Vh (concat heads)
    O1 = LN(Q + attn; g1,b1)
    O  = LN(O1 + relu(O1@Wo + bo); g2,b2)

Sharding: pure data-parallel, one batch element per NeuronCore (B=8 = 8 cores).

On-device dataflow is "feature-major": activations live in SBUF transposed
([model_dim -> 8x128 partitions, token -> free]) so every matmul chains with
no transposes.  Softmax denominators and LayerNorm stats are partition-dim
reductions done with stationary matmuls (which also broadcast the result
across partitions for free).

Precision/engines: projections and logits run bf16 into fp32 PSUM (same
1 cycle/row as fp32r, FWL weight loads, half the DMA).  The attention
weights (exp) and V are fp8e4, so the softmax denominator and A@V run as
DoubleRow matmuls at 2 MACs/cycle.  The key mask never touches the exp:
masked keys are excluded by zeroing V rows (per-partition ACT scale) and by
a 0/1 fp8 mask-column matrix standing in for all-ones in the denominator;
exp ACT ops therefore span two key-subtiles (the ACT engine, not the PE, is
the attention-phase pacer, so exp op count matters).  LayerNorm rsigma is
exp(-0.5*ln(var+eps)) and the ACT table list is steered so the whole kernel
uses the single Exp+Ln table: no mid-kernel table swaps.

Schedule: queries run in two 512-wide chunks.  Attention chunk 0 interleaves
Q-proj chunk-1 do-groups between heads; attention chunk 1 interleaves
O-proj chunk-0 groups; LayerNorm DVE work always overlaps the next PE
phase.  The PE stream stays dense end-to-end, so HAM stays at K=8/8.
"""

import numpy as np
import ml_dtypes
from contextlib import ExitStack

import concourse.bass as bass
import concourse.mybir as mybir
import concourse.tile as tile
from concourse import bacc
from concourse.bass_utils import run_bass_kernel_spmd

P = 128
NX = 1024
NY = 1024
DIM = 1024
H = 8
KO = DIM // P          # 8 partition sub-tiles of the model dim
QC = 512               # query chunk
NQC = NX // QC         # 2
F32 = mybir.dt.float32
BF16 = mybir.dt.bfloat16
F8 = mybir.dt.float8e4
DR = mybir.MatmulPerfMode.DoubleRow
AF = mybir.ActivationFunctionType
ALU = mybir.AluOpType
SCALE = 1.0 / 32.0     # 1/sqrt(DIM)
EPS = 1e-5
BF = ml_dtypes.bfloat16
F8NP = ml_dtypes.float8_e4m3
VNAMES = ("bq", "bk", "bo", "g1", "b1", "g2", "b2")
NVEC = len(VNAMES)

_TBL_PATCHED = False


def _patch_act_tables():
    """Steer every activation in this kernel to the ONE table that contains
    all of Exp/Ln/Identity/Relu/Copy (natural_log_exp_and_others) -> a single
    table load, no mid-kernel swaps.  act_func_set_id is positional into
    act_info.json, so the list ORDER must not change; instead strip this
    kernel's functions from every other table's set so the chooser picks the
    combined table at its true index."""
    global _TBL_PATCHED
    if _TBL_PATCHED:
        return
    orig = bacc.get_activation_tables

    def steered(arch):
        tabs = orig(arch)
        pref = "natural_log_exp_and_others"
        mine = {AF.Exp, AF.Ln, AF.Identity, AF.Relu, AF.Copy}
        if pref in tabs and mine <= set(tabs[pref]):
            return {k: (v if k == pref else set(v) - mine)
                    for k, v in tabs.items()}
        return tabs

    bacc.get_activation_tables = steered
    _TBL_PATCHED = True


def _build():
    _patch_act_tables()
    nc = bacc.Bacc("TRN2", target_bir_lowering=False, debug=False,
                   enable_asserts=False)

    # ---- DRAM I/O (per-core shapes) ----
    XTb = nc.dram_tensor("XTb", [DIM, NX], BF16, kind="ExternalInput").ap()
    YTb = nc.dram_tensor("YTb", [DIM, NY], BF16, kind="ExternalInput").ap()
    VPK = nc.dram_tensor("VPK", [P, KO, NVEC], F32, kind="ExternalInput").ap()
    M01 = nc.dram_tensor("M01", [P, KO], F32, kind="ExternalInput").ap()
    MM8 = nc.dram_tensor("MM8", [P, KO, P], F8, kind="ExternalInput").ap()
    Wqb = nc.dram_tensor("Wqb", [DIM, DIM], BF16, kind="ExternalInput").ap()
    Wkb = nc.dram_tensor("Wkb", [DIM, DIM], BF16, kind="ExternalInput").ap()
    Wvb = nc.dram_tensor("Wvb", [DIM, DIM], BF16, kind="ExternalInput").ap()
    Wob = nc.dram_tensor("Wob", [DIM, DIM], BF16, kind="ExternalInput").ap()
    BVB = nc.dram_tensor("bvb", [NY], BF16, kind="ExternalInput").ap()
    OT = nc.dram_tensor("OT", [DIM, NX], F32, kind="ExternalOutput").ap()

    xt3 = XTb.rearrange("(ko p) q -> p ko q", p=P)
    yt3 = YTb.rearrange("(ko p) q -> p ko q", p=P)
    wq3 = Wqb.rearrange("(ko p) d -> p ko d", p=P)
    wk3 = Wkb.rearrange("(ko p) d -> p ko d", p=P)
    wv3 = Wvb.rearrange("(ko p) d -> p ko d", p=P)
    wo3 = Wob.rearrange("(ko p) d -> p ko d", p=P)
    ot3 = OT.rearrange("(do p) q -> p do q", p=P)

    with tile.TileContext(nc) as tc:
        with ExitStack() as octx:
            const = octx.enter_context(tc.tile_pool(name="const", bufs=1))
            big = octx.enter_context(tc.tile_pool(name="big", bufs=1))
            xqp = octx.enter_context(tc.tile_pool(name="xq", bufs=1))
            wop = octx.enter_context(tc.tile_pool(name="wop", bufs=1))
            iop = tc.alloc_tile_pool(name="io1", bufs=1)

            # ---- constants (packed into 4 small DMAs) ----
            vpk = const.tile([P, KO, NVEC], F32, tag="vpk", name="vpk")
            m01_sb = const.tile([P, KO], F32, tag="m01", name="m01_sb")
            mm8 = const.tile([P, KO, P], F8, tag="mm8", name="mm8")
            bv_sb = const.tile([1, DIM], BF16, tag="v_bv", name="bv_sb")
            V = {name: i for i, name in enumerate(VNAMES)}

            def vec(name, do):
                i = V[name]
                return vpk[:, do, i:i + 1]

            ones_bf = const.tile([P, P], BF16, tag="onesbf", name="ones_bf")
            nc.vector.memset(ones_bf, 1.0)
            eps_sb = const.tile([P, 1], F32, tag="eps", name="eps_sb")
            nc.vector.memset(eps_sb, EPS)

            # ---- long-lived activation tiles ----
            ktm = big.tile([P, KO, NY], BF16, tag="ktm", name="ktm")
            vm = big.tile([P, KO, DIM], F8, tag="vm", name="vm")
            qtb = big.tile([P, KO, NX], BF16, tag="qtb", name="qtb")

            # ============ Phase 1: K, Q(c0), V projections ============
            # DMA plan: K-proj operands stream per-k on the sync queue (the
            # PE chases the arrivals); the two const DMAs slot in after the
            # first pair; the bulk later-needed tensors go as single
            # triggers on the ACT queue (parallel trigger issue, ~0.7us per
            # trigger on a queue is the real cost, not bandwidth).
            ytb = iop.tile([P, KO, NY], BF16, tag="ytb", name="ytb")
            wkt = iop.tile([P, KO, DIM], BF16, tag="wkt", name="wkt")
            wvt = iop.tile([P, KO, DIM], BF16, tag="wvt", name="wvt")
            xtb = xqp.tile([P, KO, NX], BF16, tag="xtb", name="xtb")
            wqt = xqp.tile([P, KO, DIM], BF16, tag="wqt", name="wqt")
            wot = wop.tile([P, KO, DIM], BF16, tag="wot", name="wot")
            for k in range(KO):
                nc.sync.dma_start(ytb[:, k, :], yt3[:, k, :])
                nc.sync.dma_start(wkt[:, k, :], wk3[:, k, :])
                if k == 0:
                    nc.sync.dma_start(vpk, VPK)
                    nc.sync.dma_start(bv_sb,
                                      BVB.rearrange("(one n) -> one n", one=1))
                elif k == 1:
                    nc.sync.dma_start(m01_sb, M01)
                    nc.sync.dma_start(mm8, MM8)
            nc.scalar.dma_start(xtb, xt3)
            nc.scalar.dma_start(wqt, wq3)
            nc.scalar.dma_start(wvt, wv3)
            nc.scalar.dma_start(wot, wo3)

            with tc.tile_pool(name="pp1", bufs=4, space="PSUM") as pp:
                # ---- K-proj: ktm[p,do,key] = sum_k Wk[k,d] Y^T[k,key]
                for grp in range(2):
                    pss = [pp.tile([P, 2, QC], F32, tag="ps",
                                   name=f"ps_k{grp}{i}") for i in range(4)]
                    for k in range(KO):
                        for i in range(4):
                            do = grp * 4 + i
                            for ng in range(2):
                                nc.tensor.matmul(
                                    pss[i][:, ng, :],
                                    lhsT=wkt[:, k, do * P:(do + 1) * P],
                                    rhs=ytb[:, k, ng * QC:(ng + 1) * QC],
                                    start=(k == 0), stop=(k == KO - 1))
                    for i in range(4):
                        do = grp * 4 + i
                        nc.scalar.activation(
                            ktm[:, do, :], pss[i], AF.Identity,
                            bias=vec("bk", do), scale=1.0)

                # ---- Q-proj chunk 0 (chunk 1 is interleaved into attn(0))
                for do in range(KO):
                    ps = pp.tile([P, 2, QC], F32, tag="ps", name=f"ps_q0{do}")
                    pq = ps[:, 0, :]
                    for k in range(KO):
                        nc.tensor.matmul(
                            pq, lhsT=wqt[:, k, do * P:(do + 1) * P],
                            rhs=xtb[:, k, 0:QC],
                            start=(k == 0), stop=(k == KO - 1))
                    nc.scalar.activation(
                        qtb[:, do, 0:QC], pq, AF.Identity,
                        bias=vec("bq", do), scale=1.0)

                # ---- V-proj (natural layout; per-free bias via K=1 MM;
                #      masked key rows zeroed by the per-partition scale)
                for yo in range(KO):
                    ps = pp.tile([P, 2, QC], F32, tag="ps", name=f"ps_v{yo}")
                    for k in range(KO):
                        for ng in range(2):
                            nc.tensor.matmul(
                                ps[:, ng, :],
                                lhsT=ytb[:, k, yo * P:(yo + 1) * P],
                                rhs=wvt[:, k, ng * QC:(ng + 1) * QC],
                                start=(k == 0), stop=False)
                    for ng in range(2):
                        nc.tensor.matmul(
                            ps[:, ng, :], lhsT=ones_bf[0:1, :],
                            rhs=bv_sb[:, ng * QC:(ng + 1) * QC],
                            start=False, stop=True)
                    nc.scalar.activation(
                        vm[:, yo, :], ps, AF.Identity,
                        scale=m01_sb[:, yo:yo + 1])

            # ============ Phase 2+3: per-query-chunk pipeline ============
            # io1 (ytb/wkt/wvt, 48KB/part) is dead after phase 1; release it
            # so the stage pools below reuse its address space.
            iop.release()
            stg = octx.enter_context(tc.tile_pool(name="stg", bufs=1))
            ep = octx.enter_context(tc.tile_pool(name="exp", bufs=3))
            rp = octx.enter_context(tc.tile_pool(name="rcp", bufs=2))
            sqp = octx.enter_context(tc.tile_pool(name="sq", bufs=1))
            stp = octx.enter_context(tc.tile_pool(name="st", bufs=8))
            outp = octx.enter_context(tc.tile_pool(name="out", bufs=4))
            lgp = octx.enter_context(tc.tile_pool(name="lgp", bufs=2, space="PSUM"))
            avp = octx.enter_context(tc.tile_pool(name="avp", bufs=2, space="PSUM"))
            rlp = octx.enter_context(tc.tile_pool(name="rlp", bufs=2, space="PSUM"))

            zts = [stg.tile([P, KO, QC], BF16, tag="zz", bufs=3, name=f"zt{c}")
                   for c in range(NQC)]
            z2ts = [stg.tile([P, KO, QC], BF16, tag="zz", bufs=3, name=f"z2t{c}")
                    for c in range(NQC)]
            o1ts = [stg.tile([P, KO, QC], BF16, tag="o1", bufs=2, name=f"o1t{c}")
                    for c in range(NQC)]

            def logits_head(c, h):
                # logitsT[key, q] = sum_d K^T_h[d, key] Q^T_h[d, q]; exp on
                # ACT over two key-subtiles at once, fp8 out (no mask here).
                qs = slice(c * QC, (c + 1) * QC)
                et = ep.tile([P, KO, QC], F8, tag="exp", name=f"et{c}_{h}")
                for kp in range(4):
                    pl = lgp.tile([P, 2, QC], F32, tag="lg", name=f"pl{c}{h}{kp}")
                    for j in range(2):
                        kt = 2 * kp + j
                        nc.tensor.matmul(
                            pl[:, j, :],
                            lhsT=ktm[:, h, kt * P:(kt + 1) * P],
                            rhs=qtb[:, h, qs], start=True, stop=True)
                    nc.scalar.activation(
                        et[:, 2 * kp:2 * kp + 2, :], pl, AF.Exp, scale=SCALE)
                return et

            def denom_av_head(c, h, et):
                qs = slice(c * QC, (c + 1) * QC)
                # DoubleRow fp8: contract adjacent key-subtile pairs at
                # 2 MACs/cycle.  The 0/1 mask matrix replaces all-ones in
                # the denominator; masked V rows are already zero.
                pr = rlp.tile([P, QC], F32, tag="rl", name=f"pr{c}{h}")
                for kp in range(4):
                    nc.tensor.matmul(
                        pr, lhsT=mm8[:, 2 * kp:2 * kp + 2, :],
                        rhs=et[:, 2 * kp:2 * kp + 2, :],
                        start=(kp == 0), stop=(kp == 3), perf_mode=DR)
                rc = rp.tile([P, QC], F32, tag="rc", name=f"rc{c}{h}")
                nc.vector.reciprocal_approx_fast(rc, pr)
                pa = avp.tile([P, QC], F32, tag="av", name=f"pa{c}{h}")
                for kp in range(4):
                    nc.tensor.matmul(
                        pa, lhsT=vm[:, 2 * kp:2 * kp + 2, h * P:(h + 1) * P],
                        rhs=et[:, 2 * kp:2 * kp + 2, :],
                        start=(kp == 0), stop=(kp == 3), perf_mode=DR)
                nc.vector.tensor_mul(zts[c][:, h, :], pa, rc)
                nc.vector.tensor_add(zts[c][:, h, :], zts[c][:, h, :],
                                     qtb[:, h, qs])

            def qproj1_group(do):
                ps = avp.tile([P, QC], F32, tag="av", name=f"ps_q1{do}")
                for k in range(KO):
                    nc.tensor.matmul(
                        ps, lhsT=wqt[:, k, do * P:(do + 1) * P],
                        rhs=xtb[:, k, QC:NX],
                        start=(k == 0), stop=(k == KO - 1))
                nc.scalar.activation(
                    qtb[:, do, QC:NX], ps, AF.Identity,
                    bias=vec("bq", do), scale=1.0)

            def oproj_group(c, no):
                # H^T[n, q] = sum_d Wo[d, n] O1^T[d, q]; z2 = o1 + relu(H+bo)
                # relu on DVE (tensor_scalar add+max) -- the ACT engine is
                # the attention-phase pacer, keep it exp-only there.
                ps = avp.tile([P, QC], F32, tag="av", name=f"ps_o{c}{no}")
                for k in range(KO):
                    nc.tensor.matmul(
                        ps, lhsT=wot[:, k, no * P:(no + 1) * P],
                        rhs=o1ts[c][:, k, :],
                        start=(k == 0), stop=(k == KO - 1))
                ht = sqp.tile([P, QC], BF16, tag="ht", bufs=3,
                              name=f"ht{c}{no}")
                nc.vector.tensor_scalar(
                    ht, ps, scalar1=vec("bo", no), scalar2=0.0,
                    op0=ALU.add, op1=ALU.max)
                nc.vector.tensor_add(z2ts[c][:, no, :], ht,
                                     o1ts[c][:, no, :])

            def attn_chunk(c, filler):
                # filler(i) emits one PE work-group between heads to keep
                # the PE fed while ACT drains the exps.
                prev = None
                fi = 0
                for h in range(H):
                    et = logits_head(c, h)
                    if filler is not None:
                        filler(fi); fi += 1
                    if prev is not None:
                        denom_av_head(c, h - 1, prev)
                    prev = et
                denom_av_head(c, H - 1, prev)
                return fi

            def layernorm(c, tag, in_sb, gname, bname, emit_out):
                pmu = rlp.tile([P, QC], F32, tag="rl", name=f"pmu{tag}{c}")
                ps2 = rlp.tile([P, QC], F32, tag="rl", name=f"ps2{tag}{c}")
                for do in range(KO):
                    nc.tensor.matmul(pmu, lhsT=ones_bf,
                                     rhs=in_sb[:, do, :],
                                     start=(do == 0), stop=(do == KO - 1))
                sqs = []
                for do in range(KO):
                    sq = sqp.tile([P, QC], BF16, tag="sq", bufs=8,
                                  name=f"sq{tag}{c}{do}")
                    nc.vector.tensor_mul(sq, in_sb[:, do, :], in_sb[:, do, :])
                    sqs.append(sq)
                for do in range(KO):
                    nc.tensor.matmul(ps2, lhsT=ones_bf, rhs=sqs[do],
                                     start=(do == 0), stop=(do == KO - 1))
                mu = stp.tile([P, QC], F32, tag="st", name=f"mu{tag}{c}")
                nc.vector.tensor_scalar_mul(mu, pmu, 1.0 / DIM)
                msq = stp.tile([P, QC], F32, tag="st", name=f"msq{tag}{c}")
                nc.vector.tensor_mul(msq, mu, mu)
                var = stp.tile([P, QC], F32, tag="st", name=f"var{tag}{c}")
                nc.vector.scalar_tensor_tensor(
                    var, ps2, 1.0 / DIM, msq,
                    op0=ALU.mult, op1=ALU.subtract)
                lnv = stp.tile([P, QC], F32, tag="st", name=f"lnv{tag}{c}")
                nc.scalar.activation(lnv, var, AF.Ln, bias=eps_sb, scale=1.0)
                rsig = stp.tile([P, QC], F32, tag="st", name=f"rsig{tag}{c}")
                nc.scalar.activation(rsig, lnv, AF.Exp, scale=-0.5)
                mub = stp.tile([P, QC], BF16, tag="stb", bufs=2,
                               name=f"mub{tag}{c}")
                nc.vector.tensor_copy(mub, mu)
                rsb = stp.tile([P, QC], BF16, tag="stb", bufs=2,
                               name=f"rsb{tag}{c}")
                nc.vector.tensor_copy(rsb, rsig)
                for do in range(KO):
                    t = sqp.tile([P, QC], BF16, tag="t", bufs=3,
                                 name=f"t{tag}{c}{do}")
                    nc.vector.tensor_sub(t, in_sb[:, do, :], mub)
                    nc.vector.tensor_mul(t, t, rsb)
                    emit_out(do, t)

            def ln1_chunk(c):
                def emit_o1(do, t):
                    nc.vector.tensor_scalar(
                        o1ts[c][:, do, :], t,
                        scalar1=vec("g1", do), scalar2=vec("b1", do),
                        op0=ALU.mult, op1=ALU.add)
                layernorm(c, "a", zts[c], "g1", "b1", emit_o1)

            def ln2_chunk(c):
                qs = slice(c * QC, (c + 1) * QC)

                def emit_o2(do, t):
                    o = outp.tile([P, QC], F32, tag="o", name=f"o{c}{do}")
                    nc.scalar.activation(
                        o, t, AF.Identity,
                        bias=vec("b2", do), scale=vec("g2", do))
                    nc.sync.dma_start(ot3[:, do, qs], o)
                layernorm(c, "b", z2ts[c], "g2", "b2", emit_o2)

            # attn(0) fills with Q-proj chunk-1 groups; attn(1) fills with
            # O-proj chunk-0 groups (o1t(0) is ready once LN1(0)'s DVE apply
            # drains, a couple of heads in).
            attn_chunk(0, qproj1_group)
            ln1_chunk(0)

            oq = []

            def fill1(i):
                if i >= 2:
                    oproj_group(0, i - 2)
                    oq.append(i - 2)
            attn_chunk(1, fill1)
            for no in range(len(oq), KO):
                oproj_group(0, no)
            ln1_chunk(1)
            ln2_chunk(0)
            for no in range(KO):
                oproj_group(1, no)
            ln2_chunk(1)

    nc.compile()
    return nc


_CACHE = {}


def _get_nc():
    if "nc" not in _CACHE:
        _CACHE["nc"] = _build()
    return _CACHE["nc"]


def make_in_maps(X, Y, mask, Wq, bq, Wk, bk, Wv, bv, Wo, bo, g1, b1, g2, b2):
    fb = lambda a: np.ascontiguousarray(np.asarray(a, dtype=np.float32).astype(BF))
    shared = {
        "Wqb": fb(Wq), "Wkb": fb(Wk), "Wvb": fb(Wv), "Wob": fb(Wo),
        "bvb": fb(bv),
    }
    svecs = {
        "bq": np.asarray(bq, np.float32), "bk": np.asarray(bk, np.float32),
        "bo": np.asarray(bo, np.float32), "g1": np.asarray(g1, np.float32),
        "b1": np.asarray(b1, np.float32), "g2": np.asarray(g2, np.float32),
        "b2": np.asarray(b2, np.float32),
    }
    vpk0 = np.zeros((P, KO, NVEC), np.float32)
    for i, name in enumerate(VNAMES):
        vpk0[:, :, i] = svecs[name].reshape(KO, P).T
    X = np.asarray(X, dtype=np.float32)
    Y = np.asarray(Y, dtype=np.float32)
    mask = np.asarray(mask)
    in_maps = []
    for b in range(8):
        m01 = np.where(mask[b], np.float32(0.0), np.float32(1.0))
        m01_pk = np.ascontiguousarray(m01.reshape(KO, P).T)      # [P, KO]
        mm8 = np.ascontiguousarray(
            np.broadcast_to(m01_pk[:, :, None], (P, KO, P))).astype(F8NP)
        in_maps.append({
            "XTb": np.ascontiguousarray(X[b].T.astype(BF)),
            "YTb": np.ascontiguousarray(Y[b].T.astype(BF)),
            "VPK": vpk0,
            "M01": m01_pk,
            "MM8": mm8,
            **shared,
        })
    return in_maps


def kernel(X, Y, mask, Wq, bq, Wk, bk, Wv, bv, Wo, bo, g1, b1, g2, b2,
           _trace=False):
    nc = _get_nc()
    in_maps = make_in_maps(X, Y, mask, Wq, bq, Wk, bk, Wv, bv, Wo, bo,
                           g1, b1, g2, b2)
    res = run_bass_kernel_spmd(nc, in_maps, core_ids=list(range(8)),
                               trace=_trace)
    out = np.stack([np.ascontiguousarray(res.results[b]["OT"].T)
                    for b in range(8)]).astype(np.float32)
    if _trace:
        return out, res
    return out


# revision 20
# speedup vs baseline: 1.8933x; 1.1041x over previous
"""Trainium2 Bass kernel for a masked-attention block (MAB).

Computation (per batch element):
    Q = X@Wq + bq ; K = Y@Wk + bk ; V = Y@Wv + bv
    logits = per-head Qh@Kh^T / 32, masked keys -> -inf, softmax over keys
    attn   = A @ Vh (concat heads)
    O1 = LN(Q + attn; g1,b1)
    O  = LN(O1 + relu(O1@Wo + bo); g2,b2)

Sharding: pure data-parallel, one batch element per NeuronCore (B=8 = 8 cores).

On-device dataflow is "feature-major": activations live in SBUF transposed
([model_dim -> 8x128 partitions, token -> free]) so every matmul chains with
no transposes.  Softmax denominators and LayerNorm stats are partition-dim
reductions done with stationary matmuls (which also broadcast the result
across partitions for free).

Precision/engines: projections and logits run bf16 into fp32 PSUM (same
1 cycle/row as fp32r, FWL weight loads, half the DMA).  The attention
weights (exp) and V are fp8e4, so the softmax denominator and A@V run as
DoubleRow matmuls at 2 MACs/cycle.  The key mask never touches the exp:
masked keys are excluded by zeroing V rows (per-partition ACT scale) and by
a 0/1 fp8 mask-column matrix standing in for all-ones in the denominator;
exp ACT ops therefore span two key-subtiles (the ACT engine, not the PE, is
the attention-phase pacer, so exp op count matters).  LayerNorm rsigma is
exp(-0.5*ln(var+eps)) and the ACT table list is steered so the whole kernel
uses the single Exp+Ln table: no mid-kernel table swaps.

Schedule: queries run in two 512-wide chunks.  Attention chunk 0 interleaves
Q-proj chunk-1 do-groups between heads; attention chunk 1 interleaves
O-proj chunk-0 groups; LayerNorm DVE work always overlaps the next PE
phase.  The PE stream stays dense end-to-end, so HAM stays at K=8/8.
"""

import numpy as np
import ml_dtypes
from contextlib import ExitStack

import concourse.bass as bass
import concourse.mybir as mybir
import concourse.tile as tile
from concourse import bacc
from concourse.bass_utils import run_bass_kernel_spmd

P = 128
NX = 1024
NY = 1024
DIM = 1024
H = 8
KO = DIM // P          # 8 partition sub-tiles of the model dim
QC = 512               # query chunk
NQC = NX // QC         # 2
F32 = mybir.dt.float32
BF16 = mybir.dt.bfloat16
F8 = mybir.dt.float8e4
DR = mybir.MatmulPerfMode.DoubleRow
AF = mybir.ActivationFunctionType
ALU = mybir.AluOpType
SCALE = 1.0 / 32.0     # 1/sqrt(DIM)
EPS = 1e-5
BF = ml_dtypes.bfloat16
F8NP = ml_dtypes.float8_e4m3
VNAMES = ("bq", "bk", "bo", "g1", "b1", "g2", "b2")
NVEC = len(VNAMES)

_TBL_PATCHED = False


def _patch_act_tables():
    """Steer every activation in this kernel to the ONE table that contains
    all of Exp/Ln/Identity/Relu/Copy (natural_log_exp_and_others) -> a single
    table load, no mid-kernel swaps.  act_func_set_id is positional into
    act_info.json, so the list ORDER must not change; instead strip this
    kernel's functions from every other table's set so the chooser picks the
    combined table at its true index."""
    global _TBL_PATCHED
    if _TBL_PATCHED:
        return
    orig = bacc.get_activation_tables

    def steered(arch):
        tabs = orig(arch)
        pref = "natural_log_exp_and_others"
        mine = {AF.Exp, AF.Ln, AF.Identity, AF.Relu, AF.Copy}
        if pref in tabs and mine <= set(tabs[pref]):
            return {k: (v if k == pref else set(v) - mine)
                    for k, v in tabs.items()}
        return tabs

    bacc.get_activation_tables = steered
    _TBL_PATCHED = True


def _build():
    _patch_act_tables()
    nc = bacc.Bacc("TRN2", target_bir_lowering=False, debug=False,
                   enable_asserts=False)

    # ---- DRAM I/O (per-core shapes) ----
    XTb = nc.dram_tensor("XTb", [DIM, NX], BF16, kind="ExternalInput").ap()
    YT8 = nc.dram_tensor("YT8", [DIM, NY], F8, kind="ExternalInput").ap()
    VPK = nc.dram_tensor("VPK", [P, KO, NVEC], F32, kind="ExternalInput").ap()
    M01 = nc.dram_tensor("M01", [P, KO], F32, kind="ExternalInput").ap()
    MM8 = nc.dram_tensor("MM8", [P, KO, P], F8, kind="ExternalInput").ap()
    Wqb = nc.dram_tensor("Wqb", [DIM, DIM], BF16, kind="ExternalInput").ap()
    Wk8 = nc.dram_tensor("Wk8", [DIM, DIM], F8, kind="ExternalInput").ap()
    Wv8 = nc.dram_tensor("Wv8", [DIM, DIM], F8, kind="ExternalInput").ap()
    Wob = nc.dram_tensor("Wob", [DIM, DIM], BF16, kind="ExternalInput").ap()
    BVB = nc.dram_tensor("bvb", [NY], BF16, kind="ExternalInput").ap()
    OT = nc.dram_tensor("OT", [DIM, NX], F32, kind="ExternalOutput").ap()

    xt3 = XTb.rearrange("(ko p) q -> p ko q", p=P)
    yt3 = YT8.rearrange("(ko p) q -> p ko q", p=P)
    wq3 = Wqb.rearrange("(ko p) d -> p ko d", p=P)
    wk3 = Wk8.rearrange("(ko p) d -> p ko d", p=P)
    wv3 = Wv8.rearrange("(ko p) d -> p ko d", p=P)
    wo3 = Wob.rearrange("(ko p) d -> p ko d", p=P)
    ot3 = OT.rearrange("(do p) q -> p do q", p=P)

    with tile.TileContext(nc) as tc:
        with ExitStack() as octx:
            const = octx.enter_context(tc.tile_pool(name="const", bufs=1))
            big = octx.enter_context(tc.tile_pool(name="big", bufs=1))
            xqp = octx.enter_context(tc.tile_pool(name="xq", bufs=1))
            wop = octx.enter_context(tc.tile_pool(name="wop", bufs=1))
            iop = tc.alloc_tile_pool(name="io1", bufs=1)

            # ---- constants (packed into 4 small DMAs) ----
            vpk = const.tile([P, KO, NVEC], F32, tag="vpk", name="vpk")
            m01_sb = const.tile([P, KO], F32, tag="m01", name="m01_sb")
            mm8 = const.tile([P, KO, P], F8, tag="mm8", name="mm8")
            bv_sb = const.tile([1, DIM], BF16, tag="v_bv", name="bv_sb")
            V = {name: i for i, name in enumerate(VNAMES)}

            def vec(name, do):
                i = V[name]
                return vpk[:, do, i:i + 1]

            ones_bf = const.tile([P, P], BF16, tag="onesbf", name="ones_bf")
            nc.vector.memset(ones_bf, 1.0)
            eps_sb = const.tile([P, 1], F32, tag="eps", name="eps_sb")
            nc.vector.memset(eps_sb, EPS)

            # ---- long-lived activation tiles ----
            ktm = big.tile([P, KO, NY], BF16, tag="ktm", name="ktm")
            vm = big.tile([P, KO, DIM], F8, tag="vm", name="vm")
            qtb = big.tile([P, KO, NX], BF16, tag="qtb", name="qtb")

            # ============ Phase 1: K, Q(c0), V projections ============
            # DMA plan: K-proj operands stream per-k on the sync queue (the
            # PE chases the arrivals); the two const DMAs slot in after the
            # first pair; the bulk later-needed tensors go as single
            # triggers on the ACT queue (parallel trigger issue, ~0.7us per
            # trigger on a queue is the real cost, not bandwidth).
            ytb = iop.tile([P, KO, NY], F8, tag="ytb", name="ytb")
            wkt = iop.tile([P, KO, DIM], F8, tag="wkt", name="wkt")
            wvt = iop.tile([P, KO, DIM], F8, tag="wvt", name="wvt")
            xtb = xqp.tile([P, KO, NX], BF16, tag="xtb", name="xtb")
            wqt = xqp.tile([P, KO, DIM], BF16, tag="wqt", name="wqt")
            wot = wop.tile([P, KO, DIM], BF16, tag="wot", name="wot")
            nc.scalar.dma_start(vpk, VPK)
            nc.scalar.dma_start(bv_sb, BVB.rearrange("(one n) -> one n", one=1))
            nc.scalar.dma_start(m01_sb, M01)
            for k in range(KO):
                nc.sync.dma_start(ytb[:, k, :], yt3[:, k, :])
                nc.sync.dma_start(wkt[:, k, :], wk3[:, k, :])
            nc.scalar.dma_start(xtb, xt3)
            nc.scalar.dma_start(wqt, wq3)
            nc.scalar.dma_start(wvt, wv3)
            nc.scalar.dma_start(wot, wo3)
            nc.scalar.dma_start(mm8, MM8)

            with tc.tile_pool(name="pp1", bufs=4, space="PSUM") as pp:
                # ---- K-proj: ktm[p,do,key] = sum_k Wk[k,d] Y^T[k,key]
                for grp in range(2):
                    pss = [pp.tile([P, 2, QC], F32, tag="ps",
                                   name=f"ps_k{grp}{i}") for i in range(4)]
                    for kp in range(4):
                        for i in range(4):
                            do = grp * 4 + i
                            for ng in range(2):
                                nc.tensor.matmul(
                                    pss[i][:, ng, :],
                                    lhsT=wkt[:, 2 * kp:2 * kp + 2,
                                             do * P:(do + 1) * P],
                                    rhs=ytb[:, 2 * kp:2 * kp + 2,
                                            ng * QC:(ng + 1) * QC],
                                    start=(kp == 0), stop=(kp == 3),
                                    perf_mode=DR)
                    for i in range(4):
                        do = grp * 4 + i
                        nc.scalar.activation(
                            ktm[:, do, :], pss[i], AF.Identity,
                            bias=vec("bk", do), scale=1.0)

                # ---- Q-proj chunk 0 (chunk 1 is interleaved into attn(0))
                for do in range(KO):
                    ps = pp.tile([P, 2, QC], F32, tag="ps", name=f"ps_q0{do}")
                    pq = ps[:, 0, :]
                    for k in range(KO):
                        nc.tensor.matmul(
                            pq, lhsT=wqt[:, k, do * P:(do + 1) * P],
                            rhs=xtb[:, k, 0:QC],
                            start=(k == 0), stop=(k == KO - 1))
                    nc.scalar.activation(
                        qtb[:, do, 0:QC], pq, AF.Identity,
                        bias=vec("bq", do), scale=1.0)

                # ---- V-proj (natural layout; per-free bias via K=1 MM;
                #      masked key rows zeroed by the per-partition scale)
                for yo in range(KO):
                    ps = pp.tile([P, 2, QC], F32, tag="ps", name=f"ps_v{yo}")
                    for kp in range(4):
                        for ng in range(2):
                            nc.tensor.matmul(
                                ps[:, ng, :],
                                lhsT=ytb[:, 2 * kp:2 * kp + 2,
                                         yo * P:(yo + 1) * P],
                                rhs=wvt[:, 2 * kp:2 * kp + 2,
                                        ng * QC:(ng + 1) * QC],
                                start=(kp == 0), stop=False, perf_mode=DR)
                    for ng in range(2):
                        nc.tensor.matmul(
                            ps[:, ng, :], lhsT=ones_bf[0:1, :],
                            rhs=bv_sb[:, ng * QC:(ng + 1) * QC],
                            start=False, stop=True)
                    nc.scalar.activation(
                        vm[:, yo, :], ps, AF.Identity,
                        scale=m01_sb[:, yo:yo + 1])

            # ============ Phase 2+3: per-query-chunk pipeline ============
            # io1 (ytb/wkt/wvt, 48KB/part) is dead after phase 1; release it
            # so the stage pools below reuse its address space.
            iop.release()
            stg = octx.enter_context(tc.tile_pool(name="stg", bufs=1))
            ep = octx.enter_context(tc.tile_pool(name="exp", bufs=3))
            rp = octx.enter_context(tc.tile_pool(name="rcp", bufs=2))
            sqp = octx.enter_context(tc.tile_pool(name="sq", bufs=1))
            stp = octx.enter_context(tc.tile_pool(name="st", bufs=8))
            outp = octx.enter_context(tc.tile_pool(name="out", bufs=4))
            lgp = octx.enter_context(tc.tile_pool(name="lgp", bufs=2, space="PSUM"))
            avp = octx.enter_context(tc.tile_pool(name="avp", bufs=2, space="PSUM"))
            rlp = octx.enter_context(tc.tile_pool(name="rlp", bufs=2, space="PSUM"))

            zts = [stg.tile([P, KO, QC], BF16, tag="zz", bufs=3, name=f"zt{c}")
                   for c in range(NQC)]
            z2ts = [stg.tile([P, KO, QC], BF16, tag="zz", bufs=3, name=f"z2t{c}")
                    for c in range(NQC)]
            o1ts = [stg.tile([P, KO, QC], BF16, tag="o1", bufs=2, name=f"o1t{c}")
                    for c in range(NQC)]

            def logits_head(c, h):
                # logitsT[key, q] = sum_d K^T_h[d, key] Q^T_h[d, q]; exp on
                # ACT over two key-subtiles at once, fp8 out (no mask here).
                qs = slice(c * QC, (c + 1) * QC)
                et = ep.tile([P, KO, QC], F8, tag="exp", name=f"et{c}_{h}")
                for kp in range(4):
                    pl = lgp.tile([P, 2, QC], F32, tag="lg", name=f"pl{c}{h}{kp}")
                    for j in range(2):
                        kt = 2 * kp + j
                        nc.tensor.matmul(
                            pl[:, j, :],
                            lhsT=ktm[:, h, kt * P:(kt + 1) * P],
                            rhs=qtb[:, h, qs], start=True, stop=True)
                    nc.scalar.activation(
                        et[:, 2 * kp:2 * kp + 2, :], pl, AF.Exp, scale=SCALE)
                return et

            def denom_av_head(c, h, et):
                qs = slice(c * QC, (c + 1) * QC)
                # DoubleRow fp8: contract adjacent key-subtile pairs at
                # 2 MACs/cycle.  The 0/1 mask matrix replaces all-ones in
                # the denominator; masked V rows are already zero.
                pr = rlp.tile([P, QC], F32, tag="rl", name=f"pr{c}{h}")
                for kp in range(4):
                    nc.tensor.matmul(
                        pr, lhsT=mm8[:, 2 * kp:2 * kp + 2, :],
                        rhs=et[:, 2 * kp:2 * kp + 2, :],
                        start=(kp == 0), stop=(kp == 3), perf_mode=DR)
                rc = rp.tile([P, QC], F32, tag="rc", name=f"rc{c}{h}")
                nc.vector.reciprocal_approx_fast(rc, pr)
                pa = avp.tile([P, QC], F32, tag="av", name=f"pa{c}{h}")
                for kp in range(4):
                    nc.tensor.matmul(
                        pa, lhsT=vm[:, 2 * kp:2 * kp + 2, h * P:(h + 1) * P],
                        rhs=et[:, 2 * kp:2 * kp + 2, :],
                        start=(kp == 0), stop=(kp == 3), perf_mode=DR)
                nc.vector.tensor_mul(zts[c][:, h, :], pa, rc)
                nc.vector.tensor_add(zts[c][:, h, :], zts[c][:, h, :],
                                     qtb[:, h, qs])

            def qproj1_group(do):
                ps = avp.tile([P, QC], F32, tag="av", name=f"ps_q1{do}")
                for k in range(KO):
                    nc.tensor.matmul(
                        ps, lhsT=wqt[:, k, do * P:(do + 1) * P],
                        rhs=xtb[:, k, QC:NX],
                        start=(k == 0), stop=(k == KO - 1))
                nc.scalar.activation(
                    qtb[:, do, QC:NX], ps, AF.Identity,
                    bias=vec("bq", do), scale=1.0)

            def oproj_group(c, no):
                # H^T[n, q] = sum_d Wo[d, n] O1^T[d, q]; z2 = o1 + relu(H+bo)
                # relu on DVE (tensor_scalar add+max) -- the ACT engine is
                # the attention-phase pacer, keep it exp-only there.
                ps = avp.tile([P, QC], F32, tag="av", name=f"ps_o{c}{no}")
                for k in range(KO):
                    nc.tensor.matmul(
                        ps, lhsT=wot[:, k, no * P:(no + 1) * P],
                        rhs=o1ts[c][:, k, :],
                        start=(k == 0), stop=(k == KO - 1))
                ht = sqp.tile([P, QC], BF16, tag="ht", bufs=3,
                              name=f"ht{c}{no}")
                nc.vector.tensor_scalar(
                    ht, ps, scalar1=vec("bo", no), scalar2=0.0,
                    op0=ALU.add, op1=ALU.max)
                nc.vector.tensor_add(z2ts[c][:, no, :], ht,
                                     o1ts[c][:, no, :])

            def attn_chunk(c, filler):
                # filler(i) emits one PE work-group between heads to keep
                # the PE fed while ACT drains the exps.
                prev = None
                fi = 0
                for h in range(H):
                    et = logits_head(c, h)
                    if filler is not None:
                        filler(fi); fi += 1
                    if prev is not None:
                        denom_av_head(c, h - 1, prev)
                    prev = et
                denom_av_head(c, H - 1, prev)
                return fi

            def layernorm(c, tag, in_sb, gname, bname, emit_out):
                pmu = rlp.tile([P, QC], F32, tag="rl", name=f"pmu{tag}{c}")
                ps2 = rlp.tile([P, QC], F32, tag="rl", name=f"ps2{tag}{c}")
                for do in range(KO):
                    nc.tensor.matmul(pmu, lhsT=ones_bf,
                                     rhs=in_sb[:, do, :],
                                     start=(do == 0), stop=(do == KO - 1))
                sqs = []
                for do in range(KO):
                    sq = sqp.tile([P, QC], BF16, tag="sq", bufs=8,
                                  name=f"sq{tag}{c}{do}")
                    nc.vector.tensor_mul(sq, in_sb[:, do, :], in_sb[:, do, :])
                    sqs.append(sq)
                for do in range(KO):
                    nc.tensor.matmul(ps2, lhsT=ones_bf, rhs=sqs[do],
                                     start=(do == 0), stop=(do == KO - 1))
                mu = stp.tile([P, QC], F32, tag="st", name=f"mu{tag}{c}")
                nc.vector.tensor_scalar_mul(mu, pmu, 1.0 / DIM)
                msq = stp.tile([P, QC], F32, tag="st", name=f"msq{tag}{c}")
                nc.vector.tensor_mul(msq, mu, mu)
                var = stp.tile([P, QC], F32, tag="st", name=f"var{tag}{c}")
                nc.vector.scalar_tensor_tensor(
                    var, ps2, 1.0 / DIM, msq,
                    op0=ALU.mult, op1=ALU.subtract)
                lnv = stp.tile([P, QC], F32, tag="st", name=f"lnv{tag}{c}")
                nc.scalar.activation(lnv, var, AF.Ln, bias=eps_sb, scale=1.0)
                rsig = stp.tile([P, QC], F32, tag="st", name=f"rsig{tag}{c}")
                nc.scalar.activation(rsig, lnv, AF.Exp, scale=-0.5)
                mub = stp.tile([P, QC], BF16, tag="stb", bufs=2,
                               name=f"mub{tag}{c}")
                nc.vector.tensor_copy(mub, mu)
                rsb = stp.tile([P, QC], BF16, tag="stb", bufs=2,
                               name=f"rsb{tag}{c}")
                nc.vector.tensor_copy(rsb, rsig)
                for do in range(KO):
                    t = sqp.tile([P, QC], BF16, tag="t", bufs=3,
                                 name=f"t{tag}{c}{do}")
                    nc.vector.tensor_sub(t, in_sb[:, do, :], mub)
                    nc.vector.tensor_mul(t, t, rsb)
                    emit_out(do, t)

            def ln1_chunk(c):
                def emit_o1(do, t):
                    nc.vector.tensor_scalar(
                        o1ts[c][:, do, :], t,
                        scalar1=vec("g1", do), scalar2=vec("b1", do),
                        op0=ALU.mult, op1=ALU.add)
                layernorm(c, "a", zts[c], "g1", "b1", emit_o1)

            def ln2_chunk(c):
                qs = slice(c * QC, (c + 1) * QC)

                def emit_o2(do, t):
                    o = outp.tile([P, QC], F32, tag="o", name=f"o{c}{do}")
                    nc.scalar.activation(
                        o, t, AF.Identity,
                        bias=vec("b2", do), scale=vec("g2", do))
                    nc.sync.dma_start(ot3[:, do, qs], o)
                layernorm(c, "b", z2ts[c], "g2", "b2", emit_o2)

            # attn(0) fills with Q-proj chunk-1 groups; attn(1) fills with
            # O-proj chunk-0 groups (o1t(0) is ready once LN1(0)'s DVE apply
            # drains, a couple of heads in).
            attn_chunk(0, qproj1_group)
            ln1_chunk(0)

            oq = []

            def fill1(i):
                if i >= 2:
                    oproj_group(0, i - 2)
                    oq.append(i - 2)
            attn_chunk(1, fill1)
            for no in range(len(oq), KO):
                oproj_group(0, no)
            ln1_chunk(1)
            ln2_chunk(0)
            for no in range(KO):
                oproj_group(1, no)
            ln2_chunk(1)

    nc.compile()
    return nc


_CACHE = {}


def _get_nc():
    if "nc" not in _CACHE:
        _CACHE["nc"] = _build()
    return _CACHE["nc"]


def make_in_maps(X, Y, mask, Wq, bq, Wk, bk, Wv, bv, Wo, bo, g1, b1, g2, b2):
    fb = lambda a: np.ascontiguousarray(np.asarray(a, dtype=np.float32).astype(BF))
    f8 = lambda a: np.ascontiguousarray(np.asarray(a, dtype=np.float32).astype(F8NP))
    shared = {
        "Wqb": fb(Wq), "Wk8": f8(Wk), "Wv8": f8(Wv), "Wob": fb(Wo),
        "bvb": fb(bv),
    }
    svecs = {
        "bq": np.asarray(bq, np.float32), "bk": np.asarray(bk, np.float32),
        "bo": np.asarray(bo, np.float32), "g1": np.asarray(g1, np.float32),
        "b1": np.asarray(b1, np.float32), "g2": np.asarray(g2, np.float32),
        "b2": np.asarray(b2, np.float32),
    }
    vpk0 = np.zeros((P, KO, NVEC), np.float32)
    for i, name in enumerate(VNAMES):
        vpk0[:, :, i] = svecs[name].reshape(KO, P).T
    X = np.asarray(X, dtype=np.float32)
    Y = np.asarray(Y, dtype=np.float32)
    mask = np.asarray(mask)
    in_maps = []
    for b in range(8):
        m01 = np.where(mask[b], np.float32(0.0), np.float32(1.0))
        m01_pk = np.ascontiguousarray(m01.reshape(KO, P).T)      # [P, KO]
        mm8 = np.ascontiguousarray(
            np.broadcast_to(m01_pk[:, :, None], (P, KO, P))).astype(F8NP)
        in_maps.append({
            "XTb": np.ascontiguousarray(X[b].T.astype(BF)),
            "YT8": np.ascontiguousarray(Y[b].T.astype(F8NP)),
            "VPK": vpk0,
            "M01": m01_pk,
            "MM8": mm8,
            **shared,
        })
    return in_maps


def kernel(X, Y, mask, Wq, bq, Wk, bk, Wv, bv, Wo, bo, g1, b1, g2, b2,
           _trace=False):
    nc = _get_nc()
    in_maps = make_in_maps(X, Y, mask, Wq, bq, Wk, bk, Wv, bv, Wo, bo,
                           g1, b1, g2, b2)
    res = run_bass_kernel_spmd(nc, in_maps, core_ids=list(range(8)),
                               trace=_trace)
    out = np.stack([np.ascontiguousarray(res.results[b]["OT"].T)
                    for b in range(8)]).astype(np.float32)
    if _trace:
        return out, res
    return out
